# revision 60
# baseline (speedup 1.0000x reference)
"""Trainium2 Bass kernel for a dense transformer encoder layer.

Model (fp32 reference):
    q,k,v = x@Wq+bq, x@Wk+bk, x@Wv+bv          (16 heads, d_k=64)
    attn  = softmax(q k^T / 8) v
    h     = LN(x + attn@Wo + bo)
    out   = LN(h + relu(h@W1+b1)@W2 + b2)      (ln gamma=1, beta=0)

Sharding: query-parallel over 8 cores. Core c handles batch b=c//4,
query rows (c%4)*512..+512. Each core recomputes K/V for its batch's
full 2048-token sequence (no collectives needed); host concatenates the
8 [512, 1024] output slices (the device writes feature-major; the host
transposes).

On-device layout: activations feature-major ([feature, token]) end to
end; scores transposed ([k_tok, q]) so softmax denominators come free
from a ones-column appended to V.

Precision: all five projection groups run as fp8-e4m3 DoubleRow
matmuls (weights host-prescaled by 32, 1/32 folded into evictions).
Scores/ctx stay bf16. bv is exact under softmax averaging, so V is
projected bias-free and bv@Wo+bo is folded into xres on the host.

Engine discipline learned from traces: a dma_start OCCUPIES its
issuing engine for the whole transfer, so the scalar (ACT) engine —
the attention-exp bottleneck — issues no DMAs before the output tail;
sync (HWDGE) and gpsimd (SWDGE) split all traffic as whole-tensor
transfers ordered by first use. W2 is host-repacked per-output-chunk
contiguous and streamed as one 512KB sync DMA per chunk. Attention
emits pair p's normalize+out-projection after pair p+1's score/ctx
matmuls (the PE stream is static, so this keeps the exp pipeline
dense while the all-DVE normalize drains); the attention exp stream
runs within ~6% of the 134us ScalarE floor.

LayerNorm 1 is never materialized: FFN1 consumes fp8(xres) with a K=2
rank-1 fold per chain adding (-mu)(x)colsum(W1) + std(x)b1; relu runs
in the std-scaled domain and rstd1 multiplies at the FFN2 eviction
(which also carries a rank-1 -mu1 fold); both halves of each FFN1
psum tile evict through ONE batched 1024-col relu, halving the
ACT-eviction handshakes. LN1/LN2 std comes from ACT
Sqrt (one table switch, in the post-attention trough) and rstd from
the DVE reciprocal; the fold rows ship as two plain partition-
targeted DMAs (a single partition-expanding transfer raced its
completion semaphore). LN2 statistics accumulate inside the FFN2
loop (the sum chain reads y2 f32 directly); -mu2/rstd2 broadcast via
rank-1 PE matmuls into PSUM and the normalize is two DVE ops per
feature-major chunk with per-chunk output DMAs on both idle HWDGE
queues.
"""

import os

import numpy as np
import ml_dtypes

import concourse.bass as bass
import concourse.bacc as bacc_mod
import concourse.hw_specs as hw_specs
import concourse.tile as tile
import concourse.mybir as mybir
from concourse.bass_utils import run_bass_kernel_spmd

BF16 = mybir.dt.bfloat16
F32 = mybir.dt.float32
F32R = mybir.dt.float32r
F8 = mybir.dt.float8e4
I16 = mybir.dt.int16
DR = mybir.MatmulPerfMode.DoubleRow
AF = mybir.ActivationFunctionType
OP = mybir.AluOpType

P = 128
EPS = 1e-5

# full-problem dims
D_MODEL = 1024
D_FF = 4096
N_HEADS = 16
D_K = 64
SEQ = 2048
TQ = 512          # queries per core
N_CORES = 8


def build_program(D=D_MODEL, DFF=D_FF, H=N_HEADS, S=SEQ, T=TQ):
    """Emit the per-core Bass program (SPMD: same NEFF on all cores)."""
    KO = D // P            # feature chunks of d_model
    FO = DFF // P          # feature chunks of d_ff
    TC = S // P            # key-token chunks
    HP = H // 2            # head pairs (even head on partitions 0-63, odd on 64-127)
    VW = 65                # v-aug row width: 64 v cols + ones col
    WS = min(512, D)       # weight-stream chunk width
    SC = min(512, S)       # score/psum free chunk width
    MI = WS // P
    assert H * D_K == D and TC % 4 == 0 and T <= 512

    nc = bacc_mod.Bacc()

    xT_d = nc.dram_tensor("xT", (P, D // 256, 2, S), F8, kind="ExternalInput")
    xTq_d = nc.dram_tensor("xTq", (P, D // 256, 2, T), F8, kind="ExternalInput")
    xres_d = nc.dram_tensor("xres", (D, T), F32, kind="ExternalInput")
    Wq_d = nc.dram_tensor("Wq", (P, D // 256, 2, D), F8, kind="ExternalInput")
    Wk_d = nc.dram_tensor("Wk", (P, D // 256, 2, D), F8, kind="ExternalInput")
    Wv_d = nc.dram_tensor("Wv", (P, D // 256, 2, D), F8, kind="ExternalInput")
    Wo_d = nc.dram_tensor("Wo", (D, D), BF16, kind="ExternalInput")
    # fp8 DoubleRow-interleaved FFN weights, pre-scaled by 32 on the host
    # W1 packed per-output-chunk contiguous ([P][fo][K//256][2][P]): LDW from
    # small-stride slices paces ~216ns/pass vs ~267 for 4KB-stride slices
    w1q_d = nc.dram_tensor("w1q", (P, DFF // P, D // 256, 2, P), F8,
                           kind="ExternalInput")
    # W2 packed per-output-chunk contiguous: [KO][P, DFF//256, 2, P]
    w2q_d = nc.dram_tensor("w2q", (KO, P, DFF // 256, 2, P), F8,
                           kind="ExternalInput")
    # packed per-partition biases: [bq | bk | b1 | b2] as [P, KO+KO+FO+KO]
    cpk_d = nc.dram_tensor("cpk", (P, 3 * KO + FO), F32, kind="ExternalInput")
    # rank-1 LN1-fold rows: [colsum(W1) ; b1] as [2, DFF] bf16
    c1r_d = nc.dram_tensor("c1r", (2, DFF), BF16, kind="ExternalInput")
    # output is written feature-major [D, T]; the host transposes
    out_d = nc.dram_tensor("out", (D, T), F32, kind="ExternalOutput")

    def wr(w):  # [K, M] weight dram -> [P, K//P, M] partition-chunked view
        return w[:, :].rearrange("(o p) m -> p o m", p=P)

    with tile.TileContext(nc) as tc:
        with (
            tc.tile_pool(name="sb", bufs=1) as sb,
            tc.tile_pool(name="ps", bufs=1, space="PSUM") as ps,
        ):
            # ---- phase A: projections; q first so the PE starts early ----
            KO2 = KO // 2
            xTq = sb.tile([P, KO2, 2, T], F8, tag="mid", bufs=2, name="xTq")
            qT = sb.tile([P, KO, T], BF16, tag="mid", bufs=2, name="qT")
            # PE warm-up: HAM releases the clock gate after ~3.4us of activity.
            # Junk matmuls on not-yet-written qT (never read back) span the
            # first input DMAs so the real chains start at full clock.
            wps = ps.tile([P, 2, T], F32, tag="mm", bufs=2, name="wps")
            for i in range(5):
                nc.tensor.matmul(wps[:, 0, :], lhsT=qT[:, 0, 0:P], rhs=qT[:, 0, :],
                                 start=(i == 0), stop=(i == 4))
            xT = sb.tile([P, KO2, 2, S], F8, tag="big", bufs=3, name="xT")
            kT = sb.tile([P, KO, S], BF16, tag="big", bufs=3, name="kT")
            vAug = sb.tile([P, TC, H, VW], BF16, tag="big", bufs=3, name="vAug")
            cpk = sb.tile([P, 3 * KO + FO], F32, name="cpk")
            bq_t, bk_t = cpk[:, 0:KO], cpk[:, KO:2 * KO]
            b2_t = cpk[:, 2 * KO + FO:]
            c1r_t = sb.tile([2, DFF], BF16, name="c1r_t")
            onesf_1p = sb.tile([1, P], F32, name="onesf_1p")
            onesf_bcol = sb.tile([P, 1], F32, name="onesf_bcol")
            ones32_1p = sb.tile([1, P], BF16, name="ones32_1p")
            ones_bcol = sb.tile([P, 1], BF16, name="ones_bcol")
            eps_t = sb.tile([1, 1], F32, name="eps_t")

            # startup DMAs: scalar (ACT) issues NO dmas — a dma_start occupies
            # its engine for the whole transfer and ACT is the exp bottleneck.
            # gpsimd (SWDGE) and sync (HWDGE) split the traffic, ordered by
            # first use.
            w4 = S // 4
            wq0 = sb.tile([P, KO2, 2, WS], F8, tag="wst", bufs=3, name="wq0")
            wq1 = sb.tile([P, KO2, 2, WS], F8, tag="wst", bufs=3, name="wq1")
            nc.gpsimd.dma_start(xTq, xTq_d[:, :, :, :])
            nc.sync.dma_start(wq0, Wq_d[:, :, :, 0:WS])
            nc.gpsimd.dma_start(cpk, cpk_d[:, :])
            nc.sync.dma_start(wq1, Wq_d[:, :, :, WS:2 * WS])
            nc.gpsimd.dma_start(xT[:, :, :, 0:w4], xT_d[:, :, :, 0:w4])
            nc.gpsimd.dma_start(xT[:, :, :, 2 * w4:3 * w4],
                                xT_d[:, :, :, 2 * w4:3 * w4])
            nc.vector.memset(onesf_1p, 1.0)
            nc.vector.memset(onesf_bcol, 1.0)
            nc.vector.memset(ones32_1p, 32.0)
            nc.vector.memset(ones_bcol, 1.0)
            nc.vector.memset(eps_t, EPS)
            # touch ACT immediately so its one-time table load (~2.7us with
            # drain) runs during the initial DMA wait instead of stalling the
            # first Q eviction
            nc.scalar.activation(eps_t, eps_t, AF.Copy, bias=0.0, scale=1.0)

            for mo2 in range(D // WS):
                wt = (wq0, wq1)[mo2]
                for mi in range(0, MI, 2):
                    pst = ps.tile([P, 2, T], F32, tag="mm", bufs=2, name=f"qp{mo2}_{mi}")
                    for half in range(2):
                        mo = mo2 * MI + mi + half
                        msl = slice((mi + half) * P, (mi + half + 1) * P)
                        for o2 in range(KO2):
                            nc.tensor.matmul(
                                pst[:, half, :],
                                lhsT=wt[:, o2, :, msl],
                                rhs=xTq[:, o2, :, :], perf_mode=DR,
                                start=(o2 == 0), stop=(o2 == KO2 - 1))
                        nc.scalar.activation(qT[:, mo, :], pst[:, half, :], AF.Identity,
                                             bias=bq_t[:, mo:mo + 1], scale=1.0 / 32)

            # k^T [D, S]: two token-chunk chains per psum tile, one batched evict
            for mo2 in range(D // WS):
                wt = sb.tile([P, KO2, 2, WS], F8, tag="wst", bufs=3, name=f"wk{mo2}")
                nc.sync.dma_start(wt, Wk_d[:, :, :, mo2 * WS:(mo2 + 1) * WS])
                if mo2 == 0:
                    nc.sync.dma_start(xT[:, :, :, w4:2 * w4],
                                      xT_d[:, :, :, w4:2 * w4])
                    nc.sync.dma_start(xT[:, :, :, 3 * w4:4 * w4],
                                      xT_d[:, :, :, 3 * w4:4 * w4])
                for mi in range(MI):
                    mo = mo2 * MI + mi
                    n_ch = S // SC
                    for nc2 in range((n_ch + 1) // 2):
                        w = min(2, n_ch - nc2 * 2)
                        pst = ps.tile([P, 2, SC], F32, tag="mm", bufs=2,
                                      name=f"kp{mo}_{nc2}")
                        for half in range(w):
                            ncc = nc2 * 2 + half
                            for o2 in range(KO2):
                                nc.tensor.matmul(pst[:, half, :],
                                                 lhsT=wt[:, o2, :, mi * P:(mi + 1) * P],
                                                 rhs=xT[:, o2, :, ncc * SC:(ncc + 1) * SC],
                                                 perf_mode=DR,
                                                 start=(o2 == 0), stop=(o2 == KO2 - 1))
                        nc.scalar.activation(
                            kT[:, mo, nc2 * 2 * SC:nc2 * 2 * SC + w * SC],
                            pst[:, 0:w, :], AF.Identity,
                            bias=bk_t[:, mo:mo + 1], scale=1.0 / 32)

            xres = sb.tile([P, KO, T], F32, tag="res", bufs=2, name="xres")

            # v token-major [S, D] with appended ones column per head:
            # vAug[p, tc, h, 0:64] = v[tc*128+p, h*64:(h+1)*64],  vAug[.., 64] = 1
            # (bv is folded into xres on the host: softmax weights sum to 1)
            # Evictions run on DVE so ACT stays exp-only during attention.
            nc.vector.memset(vAug[:, :, :, D_K:D_K + 1], 1.0)
            wv0 = sb.tile([P, KO2, 2, WS], F8, tag="wst", bufs=3, name="wv0")
            wv1 = sb.tile([P, KO2, 2, WS], F8, tag="wst", bufs=3, name="wv1")
            nc.sync.dma_start(wv0, Wv_d[:, :, :, 0:WS])
            nc.sync.dma_start(wv1, Wv_d[:, :, :, WS:2 * WS])
            # xres (2MB, first needed ~10us into attention) rides the SYNC
            # ring BEHIND all projection weights: engine DMA queues run ahead
            # of program order, so on gpsimd this transfer fired mid-K-phase
            # and starved the weight streams of aggregate SDMA bandwidth
            # (wk0 was observed arriving 17us after issue)
            nc.sync.dma_start(xres, xres_d[:, :].rearrange("(o p) t -> p o t", p=P))
            nh = WS // D_K

            def v_chunk(no2, tc_, wt):
                pfull = ps.tile([P, 2, SC], F32, tag="mm", bufs=2,
                                name=f"vp{no2}_{tc_}")
                pst = pfull[:, 0, :WS]
                for o2 in range(KO2):
                    nc.tensor.matmul(pst,
                                     lhsT=xT[:, o2, :, tc_ * P:(tc_ + 1) * P],
                                     rhs=wt[:, o2, :, :], perf_mode=DR,
                                     start=(o2 == 0), stop=(o2 == KO2 - 1))
                nc.scalar.activation(
                    vAug[:, tc_, no2 * nh:(no2 + 1) * nh, 0:D_K],
                    pst.rearrange("p (h d) -> p h d", d=D_K),
                    AF.Copy, bias=0.0, scale=1.0 / 32)

            for tc_ in range(TC):
                v_chunk(0, tc_, wv0)
            for tc_ in range(TC):
                v_chunk(1, tc_, wv1)

            # W1's 4MB preload is gated on xT's big-ring slot being released
            # by the last V chain, so it cannot contend with the startup
            nc.gpsimd.dma_start(c1r_t, c1r_d[:, :])
            w1q = sb.tile([P, DFF // P, D // 256, 2, P], F8, tag="big", bufs=3,
                          name="w1q")
            nc.gpsimd.dma_start(w1q, w1q_d[:, :, :, :])

            def bcast_prep(t):
                # stream_shuffle streams all 32 input lanes; zero the quadrant
                # BEFORE the row-0 write so nothing is read uninitialized.
                nc.vector.memset(t[0:32, :], 0.0)

            def bcast_from_row0(t, rows=128):
                """Replicate t[0:1, :] (SBUF) to partitions 0..rows, DVE-only."""
                nc.vector.stream_shuffle(t[32:64, :], t[0:32, :], mask=[0] * 32)
                nc.vector.tensor_copy(t[0:32, :], t[32:64, :])
                if rows > 64:
                    nc.vector.tensor_copy(t[64:96, :], t[32:64, :])
                    nc.vector.tensor_copy(t[96:128, :], t[32:64, :])

            # ---- phase B: attention with fused out-projection ----
            # Per kc chunk both heads of a pair land in ONE [P, 2, T] psum tile
            # so a single ACT exp covers 1024 elements. ctx row 64 = softmax
            # denominator (ones column of vAug). ctx accumulators live in a
            # 3-deep "cps" ring (pair p+1 never waits on pair p's normalize);
            # pair p's out-projection matmuls interleave into pair p+1's kc
            # loop against a dedicated 1-bank "op" ring, each eviction a DVE
            # add into xres.
            def norm_muls(hp, cpsA, cpsB):
                ctxp = sb.tile([P, T], BF16, tag="ctxp", bufs=2, name=f"cx{hp}")
                for h, cps in ((2 * hp, cpsA), (2 * hp + 1, cpsB)):
                    base = D_K * (h % 2)
                    bcs = sb.tile([P, T], F32, tag="scr", bufs=3, name=f"bc{h}")
                    bcast_prep(bcs)
                    # custom-DVE ops mis-read PSUM: stage the denominator row
                    # into SBUF first, then reciprocal in place.
                    nc.vector.tensor_copy(bcs[0:1, :], cps[D_K:D_K + 1, :])
                    nc.vector.reciprocal_approx_fast(bcs[0:1, :], bcs[0:1, :])
                    bcast_from_row0(bcs, rows=64)
                    nc.vector.tensor_mul(ctxp[base:base + D_K, :],
                                         cps[0:D_K, :], bcs[0:D_K, :])
                return ctxp

            def emit_op(hp, ctxp, wo_t, mo):
                # acc-ring op tile: during pair p's loop the ring holds the
                # two open ctx accumulators plus two free slots (pair p-1's,
                # released by its norm_muls at the top of pair p); the
                # allocator hands out free slots, so these never touch the
                # open accumulators or the score-tile (mm) ring.
                op = ps.tile([P, T], F32, tag="acc", bufs=4, name=f"o{hp}_{mo}")
                nc.tensor.matmul(op, lhsT=wo_t[:, mo * P:(mo + 1) * P],
                                 rhs=ctxp, start=True, stop=True)
                nc.vector.tensor_add(xres[:, mo, :], op, xres[:, mo, :])

            def op_pair(wo_t, ctxp, mg, name):
                """Two out-proj chunks into one mm-ring tile + one DVE add."""
                op2 = ps.tile([P, 2, T], F32, tag="mm", bufs=2, name=name)
                for half in range(2):
                    mo = 2 * mg + half
                    nc.tensor.matmul(op2[:, half, :],
                                     lhsT=wo_t[:, mo * P:(mo + 1) * P],
                                     rhs=ctxp, start=True, stop=True)
                xr2 = xres[:, 2 * mg:2 * mg + 2, :]
                nc.vector.tensor_add(xr2, op2, xr2)
                return xr2

            # Pair p-1's normalize (all-DVE) is emitted at pair p's top and
            # runs under the first ~6 exps; its 8 out-proj matmuls then
            # interleave one-per-2-kc from kc=6 (5 in-loop, 3 at the
            # boundary), filling PE slack in the ACT-bound steady state so
            # only pair 7's normalize+out-proj remains after the last exp.
            pend = None
            opnd = None
            for hp in range(HP):
                hA, hB = 2 * hp, 2 * hp + 1
                wo_t = sb.tile([P, D], BF16, tag="wo", bufs=2, name=f"wo{hp}")
                nc.sync.dma_start(wo_t, wr(Wo_d)[:, hp, :])
                cpsA = ps.tile([P, T], F32, tag="acc", bufs=4, name=f"cA{hp}")
                cpsB = ps.tile([P, T], F32, tag="acc", bufs=4, name=f"cB{hp}")
                if pend is not None:
                    opnd = (pend[0], norm_muls(*pend[:3]), pend[3])
                for kc in range(TC):
                    s2 = ps.tile([P, 2, T], F32, tag="mm", bufs=2, name=f"s{hp}_{kc}")
                    nc.tensor.matmul(s2[:, 0, :],
                                     lhsT=kT[0:D_K, hp, kc * P:(kc + 1) * P],
                                     rhs=qT[0:D_K, hp, :], start=True, stop=True)
                    nc.tensor.matmul(s2[:, 1, :],
                                     lhsT=kT[D_K:P, hp, kc * P:(kc + 1) * P],
                                     rhs=qT[D_K:P, hp, :], start=True, stop=True)
                    e2 = sb.tile([P, 2, T], BF16, tag="e", bufs=4,
                                 name=f"e{hp}_{kc}")
                    nc.scalar.activation(e2, s2, AF.Exp, scale=0.125)
                    nc.tensor.matmul(cpsA[0:D_K + 1, :],
                                     lhsT=vAug[:, kc, hA, 0:D_K + 1],
                                     rhs=e2[:, 0, :],
                                     start=(kc == 0), stop=(kc == TC - 1))
                    nc.tensor.matmul(cpsB[0:D_K + 1, :],
                                     lhsT=vAug[:, kc, hB, 0:D_K + 1],
                                     rhs=e2[:, 1, :],
                                     start=(kc == 0), stop=(kc == TC - 1))
                    if opnd is not None and ((kc >= 6 and kc % 2 == 0)
                                             or kc == 15):
                        j = 5 if kc == 15 else (kc - 6) // 2
                        emit_op(opnd[0], opnd[1], opnd[2], j)
                if opnd is not None:
                    for mo in (6, 7):
                        emit_op(opnd[0], opnd[1], opnd[2], mo)
                    opnd = None
                pend = (hp, cpsA, cpsB, wo_t)

            # ---- pair 7 out-projection with LN1 stats interleaved ----
            # No exp pressure remains: per 2 chunks, two op matmuls into one
            # mm-ring tile, ONE DVE add, ONE 1024-col fp8 quantize, then the
            # LN1 stat chains (LN1 itself is folded into FFN1).
            ctxp7, wo7 = norm_muls(7, cpsA, cpsB), wo_t
            xres_q = sb.tile([P, KO // 2, 2, T], F8, tag="mid", bufs=2, name="xres_q")
            sum1_ps = ps.tile([1, T], F32, tag="acc", bufs=4, name="sum1_ps")
            ssq1_ps = ps.tile([1, T], F32, tag="acc", bufs=4, name="ssq1_ps")
            for mg in range(KO // 2):
                xr2 = op_pair(wo7, ctxp7, mg, f"o7_{mg}")
                xq2 = xres_q[:, mg, :, :]
                nc.scalar.activation(xq2, xr2, AF.Copy, bias=0.0, scale=1.0)
                ysq = sb.tile([P, 2, T], BF16, tag="ysq", bufs=2, name=f"ys1_{mg}")
                nc.vector.tensor_mul(ysq, xq2, xq2)
                for half in range(2):
                    mo = 2 * mg + half
                    nc.tensor.matmul(sum1_ps, lhsT=ones_bcol, rhs=xq2[:, half, :],
                                     start=(mo == 0), stop=(mo == KO - 1))
                    nc.tensor.matmul(ssq1_ps, lhsT=ones_bcol, rhs=ysq[:, half, :],
                                     start=(mo == 0), stop=(mo == KO - 1))

            # ---- phase C: LN1 rows (stats only; LN1 is folded into FFN) ----
            # std via ACT Sqrt (one table switch, stays loaded through LN2),
            # rstd via the standard DVE reciprocal.
            mu1f = sb.tile([1, T], F32, tag="lns", bufs=4, name="mu1f")
            nc.scalar.activation(mu1f, sum1_ps, AF.Copy, bias=0.0, scale=1.0 / D)
            var1 = sb.tile([1, T], F32, tag="lns", bufs=4, name="var1")
            nc.vector.tensor_mul(var1, mu1f, mu1f)
            nc.vector.scalar_tensor_tensor(out=var1, in0=ssq1_ps, scalar=1.0 / D,
                                           in1=var1, op0=OP.mult, op1=OP.subtract)
            std1 = sb.tile([1, T], F32, tag="lns", bufs=4, name="std1")
            nc.scalar.activation(std1, var1, AF.Sqrt, bias=eps_t[0:1, 0:1],
                                 scale=1.0)
            rstd1_sb = sb.tile([P, T], F32, tag="scr", bufs=3, name="rstd1_sb")
            bcast_prep(rstd1_sb)
            nc.vector.reciprocal(rstd1_sb[0:1, :], std1)
            bcast_from_row0(rstd1_sb)
            # pack [-mu ; std] contiguously in one partition-0 stage tile and
            # ship both rows with a single partition-crossing DMA (engine APs
            # can't write partition 1, but DMA descriptors can)
            rstage = sb.tile([1, 2, T], BF16, tag="lns", bufs=4, name="rstage")
            nc.scalar.activation(rstage[0:1, 0, :], sum1_ps, AF.Copy, bias=0.0,
                                 scale=-1.0 / D)
            nc.scalar.activation(rstage[0:1, 1, :], std1, AF.Copy, bias=0.0,
                                 scale=1.0)
            negmu_row = rstage[0:1, 0, :]
            rmix = sb.tile([2, T], BF16, tag="lns", bufs=4, name="rmix")
            # two plain partition-targeted DMAs: the single partition-
            # expanding transfer was observed to race its completion
            # semaphore (stride-4 column groups landing late)
            nc.sync.dma_start(rmix[0:1, :], rstage[0:1, 0, :])
            nc.sync.dma_start(rmix[1:2, :], rstage[0:1, 1, :])

            # ---- phase D: FFN1 + relu (std-scaled domain, no bias on ACT) ----
            # fp8 DoubleRow: 4 K=256 matmuls per chain; psum carries 32x the
            # true value (host-scaled weights), rescaled at the relu eviction.
            rT = sb.tile([P, FO // 2, 2, T], F8, tag="big", bufs=3, name="rT")
            for fo2 in range(DFF // WS):
                for fi in range(0, MI, 2):
                    pst = ps.tile([P, 2, T], F32, tag="mm", bufs=2, name=f"zp{fo2}_{fi}")
                    for half in range(2):
                        fo = fo2 * MI + fi + half
                        for o2 in range(KO // 2):
                            nc.tensor.matmul(pst[:, half, :],
                                             lhsT=w1q[:, fo, o2, :, :],
                                             rhs=xres_q[:, o2, :, :],
                                             perf_mode=DR,
                                             start=(o2 == 0), stop=False)
                        # K=2 rank-1 fold: (-mu)(x)colsum(W1) + std(x)b1
                        nc.tensor.matmul(pst[:, half, :],
                                         lhsT=c1r_t[0:2, fo * P:(fo + 1) * P],
                                         rhs=rmix[0:2, :], start=False, stop=True)
                    # both halves land in the same rT [., fo//2, 0:2, .]
                    # slice: ONE batched 1024-col relu eviction per psum tile
                    fo = fo2 * MI + fi
                    nc.scalar.activation(rT[:, fo // 2, :, :], pst,
                                         AF.Relu, bias=0.0, scale=1.0 / 32)

            # ---- phase E: FFN2 + residual + LN2 stats (interleaved) ----
            y2 = sb.tile([P, KO, T], F32, tag="res", bufs=2, name="y2")
            sum2_ps = ps.tile([1, T], F32, tag="acc", bufs=4, name="sum2_ps")
            ssq2_ps = ps.tile([1, T], F32, tag="acc", bufs=4, name="ssq2_ps")
            for mo in range(KO):
                pfull = ps.tile([P, 2, T], F32, tag="mm", bufs=2, name=f"fp{mo}")
                pst = pfull[:, 0, :]
                w2t = sb.tile([P, DFF // 256, 2, P], F8, tag="w2", bufs=2,
                              name=f"w2_{mo}")
                nc.sync.dma_start(w2t, w2q_d[mo])
                for ki in range(DFF // 256):
                    nc.tensor.matmul(pst, lhsT=w2t[:, ki, :, :],
                                     rhs=rT[:, ki, :, :], perf_mode=DR,
                                     start=(ki == 0), stop=False)
                # rank-1: subtract 32*mu1 (broadcast over features) in-psum
                nc.tensor.matmul(pst, lhsT=ones32_1p[0:1, :],
                                 rhs=negmu_row, start=False, stop=True)
                # y2 = rstd1*(ffpsum/32 + xres - mu1) + b2
                nc.vector.scalar_tensor_tensor(out=y2[:, mo, :], in0=pst,
                                               scalar=1.0 / 32, in1=xres[:, mo, :],
                                               op0=OP.mult, op1=OP.add)
                nc.vector.tensor_mul(y2[:, mo, :], y2[:, mo, :], rstd1_sb)
                nc.vector.tensor_scalar_add(y2[:, mo, :], y2[:, mo, :],
                                            b2_t[:, mo:mo + 1])
                # LN2 stats accumulate as chunks complete; the sum chain
                # consumes y2 (f32) directly — no bf16 staging copy
                ysq2 = sb.tile([P, T], BF16, tag="ysq", bufs=2, name=f"ys2_{mo}")
                nc.scalar.activation(ysq2, y2[:, mo, :], AF.Square, bias=0.0,
                                     scale=1.0)
                nc.tensor.matmul(sum2_ps, lhsT=onesf_bcol, rhs=y2[:, mo, :],
                                 start=(mo == 0), stop=(mo == KO - 1))
                nc.tensor.matmul(ssq2_ps, lhsT=ones_bcol, rhs=ysq2,
                                 start=(mo == 0), stop=(mo == KO - 1))

            # ---- phase F: LN2 normalize feature-major + store ----
            # -mu2 and rstd2 rows are PE-broadcast to all 128 partitions via
            # rank-1 matmuls into PSUM; the normalize is then two DVE
            # tensor_tensor ops per chunk and the output DMAs feature-major
            # (the host transposes).
            negmu2 = sb.tile([1, T], F32, tag="lns", bufs=4, name="negmu2")
            nc.scalar.activation(negmu2, sum2_ps, AF.Copy, bias=0.0,
                                 scale=-1.0 / D)
            t2m = sb.tile([1, T], F32, tag="lns", bufs=4, name="t2m")
            nc.vector.tensor_mul(t2m, negmu2, negmu2)
            var2 = sb.tile([1, T], F32, tag="lns", bufs=4, name="var2")
            nc.vector.scalar_tensor_tensor(out=var2, in0=ssq2_ps, scalar=1.0 / D,
                                           in1=t2m, op0=OP.mult, op1=OP.subtract)
            std2 = sb.tile([1, T], F32, tag="lns", bufs=4, name="std2")
            nc.scalar.activation(std2, var2, AF.Sqrt, bias=eps_t[0:1, 0:1],
                                 scale=1.0)
            rstd2 = sb.tile([1, T], F32, tag="lns", bufs=4, name="rstd2")
            nc.vector.reciprocal(rstd2, std2)
            nm2_ps = ps.tile([P, T], F32, tag="acc", bufs=4, name="nm2_ps")
            rs2_ps = ps.tile([P, T], F32, tag="acc", bufs=4, name="rs2_ps")
            nc.tensor.matmul(nm2_ps, lhsT=onesf_1p, rhs=negmu2,
                             start=True, stop=True)
            nc.tensor.matmul(rs2_ps, lhsT=onesf_1p, rhs=rstd2,
                             start=True, stop=True)
            out_r = out_d[:, :].rearrange("(o p) t -> p o t", p=P)
            for mo in range(KO):
                y2n = sb.tile([P, T], F32, tag="scr", bufs=3, name=f"y2n_{mo}")
                nc.vector.tensor_add(y2n, y2[:, mo, :], nm2_ps)
                nc.vector.tensor_mul(y2n, y2n, rs2_ps)
                # scalar (ACT) is idle at the tail; both HWDGE queues share
                # the output so the last chunk lands as early as possible
                (nc.sync, nc.scalar)[mo % 2].dma_start(out_r[:, mo, :], y2n)

    nc.finalize()
    return nc


def _maybe_enable_ldw_opt():
    if os.environ.get("BASS_LDW_OPT") != "1":
        return
    import concourse.bass_utils as _bu
    if getattr(_bu, "_ldw_opt_patched", False):
        return
    _orig = _bu.run_command

    def _patched(argv, **kw):
        argv = ["--enable-ldw-opt=true" if a == "--enable-ldw-opt=false" else a
                for a in argv]
        return _orig(argv, **kw)

    _bu.run_command = _patched
    _bu._ldw_opt_patched = True


_maybe_enable_ldw_opt()

_PROG = None
_last_results = None


def _get_prog():
    global _PROG
    if _PROG is None:
        _PROG = build_program()
    return _PROG


def pack_consts(bq, bk, b1, b2, KO=D_MODEL // P, FO=D_FF // P):
    cols = []
    for vec, n in ((bq, KO), (bk, KO), (b1, FO), (b2, KO)):
        cols.append(np.asarray(vec, np.float32).reshape(n, P).T)  # [P, n]
    return np.ascontiguousarray(np.concatenate(cols, axis=1))


def make_in_maps(x, Wq, bq, Wk, bk, Wv, bv, Wo, bo, W1, b1, W2, b2,
                 ln1_g, ln1_b, ln2_g, ln2_b):
    bf = ml_dtypes.bfloat16
    f32 = np.float32
    x = np.asarray(x, f32)
    f8 = ml_dtypes.float8_e4m3
    W1f = np.asarray(W1, f32)
    c1r = (32.0 * np.stack([W1f.sum(axis=0), np.asarray(b1, f32)])).astype(bf)

    def pack_dr(w):  # [K, M] -> [P, K//256, 2, M] fp8, pre-scaled by 32
        K, M = w.shape
        wi = (np.asarray(w, f32) * 32.0).reshape(K // 256, 2, P, M)
        return np.ascontiguousarray(wi.transpose(2, 0, 1, 3).astype(f8))

    w2q = pack_dr(np.asarray(W2, f32))          # [P, DFF//256, 2, D]
    # repack per-output-chunk contiguous: [KO, P, DFF//256, 2, P]
    w2q = np.ascontiguousarray(
        w2q.reshape(P, D_FF // 256, 2, D_MODEL // P, P).transpose(3, 0, 1, 2, 4))

    shared = {
        "Wq": pack_dr(np.asarray(Wq, f32)),
        "Wk": pack_dr(np.asarray(Wk, f32)),
        "Wv": pack_dr(np.asarray(Wv, f32)),
        "Wo": np.ascontiguousarray(np.asarray(Wo, f32).astype(bf)),
        "w1q": np.ascontiguousarray(
            pack_dr(W1f).reshape(P, D_MODEL // 256, 2, D_FF // P, P)
            .transpose(0, 3, 1, 2, 4)),
        "w2q": w2q,
        "cpk": pack_consts(bq, bk, b1, b2),
        "c1r": np.ascontiguousarray(c1r),
    }
    # bv is invariant under softmax averaging: attn(v + bv) = attn(v) + bv,
    # so fold bv@Wo + bo into the residual once on the host (exact, f32).
    res_bias = (np.asarray(bv, f32) @ np.asarray(Wo, f32)
                + np.asarray(bo, f32))

    def pack_act(a):  # [D, Ntok] -> [P, D//256, 2, Ntok] fp8 interleaved
        Dd, Nt = a.shape
        return np.ascontiguousarray(
            a.reshape(Dd // 256, 2, P, Nt).transpose(2, 0, 1, 3).astype(f8))

    in_maps = []
    xT_by_batch = [np.ascontiguousarray(x[b].T) for b in range(x.shape[0])]
    xTq_by_batch = [pack_act(t) for t in xT_by_batch]
    for c in range(N_CORES):
        b, q0 = c // 4, (c % 4) * TQ
        xslice = xT_by_batch[b][:, q0:q0 + TQ]
        m = dict(shared)
        m["xT"] = xTq_by_batch[b]
        m["xTq"] = np.ascontiguousarray(xTq_by_batch[b][:, :, :, q0:q0 + TQ])
        m["xres"] = np.ascontiguousarray(xslice + res_bias[:, None])
        in_maps.append(m)
    return in_maps


def kernel(**inputs):
    global _last_results
    nc = _get_prog()
    in_maps = make_in_maps(**inputs)
    res = run_bass_kernel_spmd(nc, in_maps, core_ids=list(range(N_CORES)),
                               tmpdir=os.environ.get("BASS_KERNEL_TMPDIR"))
    _last_results = res
    x = np.asarray(inputs["x"])
    B, S, D = x.shape
    out = np.empty((B, S, D), np.float32)
    for c in range(N_CORES):
        b, q0 = c // 4, (c % 4) * TQ
        out[b, q0:q0 + TQ, :] = res.results[c]["out"].T
    return out


# revision 61
# speedup vs baseline: 1.1893x; 1.1893x over previous
"""Trainium2 Bass kernel for a dense transformer encoder layer.

Model (fp32 reference):
    q,k,v = x@Wq+bq, x@Wk+bk, x@Wv+bv          (16 heads, d_k=64)
    attn  = softmax(q k^T / 8) v
    h     = LN(x + attn@Wo + bo)
    out   = LN(h + relu(h@W1+b1)@W2 + b2)      (ln gamma=1, beta=0)

Sharding: query-parallel over 8 cores. Core c handles batch b=c//4,
query rows (c%4)*512..+512. Each core recomputes K/V for its batch's
full 2048-token sequence (no collectives needed); host concatenates the
8 [512, 1024] output slices (the device writes feature-major; the host
transposes).

On-device layout: activations feature-major ([feature, token]) end to
end; scores transposed ([k_tok, q]) so softmax denominators come free
from a ones-column appended to V.

Precision: all five projection groups run as fp8-e4m3 DoubleRow
matmuls (weights host-prescaled by 32, 1/32 folded into evictions).
Scores/ctx stay bf16. bv is exact under softmax averaging, so V is
projected bias-free and bv@Wo+bo is folded into xres on the host.

Engine discipline learned from traces: a dma_start OCCUPIES its
issuing engine for the whole transfer, so the scalar (ACT) engine —
the attention-exp bottleneck — issues no DMAs before the output tail;
sync (HWDGE) and gpsimd (SWDGE) split all traffic as whole-tensor
transfers ordered by first use. W2 is host-repacked per-output-chunk
contiguous and streamed as one 512KB sync DMA per chunk. Attention
emits pair p's normalize+out-projection after pair p+1's score/ctx
matmuls (the PE stream is static, so this keeps the exp pipeline
dense while the all-DVE normalize drains); the attention exp stream
runs within ~6% of the 134us ScalarE floor.

LayerNorm 1 is never materialized: FFN1 consumes fp8(xres) with a K=2
rank-1 fold per chain adding (-mu)(x)colsum(W1) + std(x)b1; relu runs
in the std-scaled domain and rstd1 multiplies at the FFN2 eviction
(which also carries a rank-1 -mu1 fold); both halves of each FFN1
psum tile evict through ONE batched 1024-col relu, halving the
ACT-eviction handshakes. LN1/LN2 std comes from ACT
Sqrt (one table switch, in the post-attention trough) and rstd from
the DVE reciprocal; the fold rows ship as two plain partition-
targeted DMAs (a single partition-expanding transfer raced its
completion semaphore). LN2 statistics accumulate inside the FFN2
loop (the sum chain reads y2 f32 directly); -mu2/rstd2 broadcast via
rank-1 PE matmuls into PSUM and the normalize is two DVE ops per
feature-major chunk with per-chunk output DMAs on both idle HWDGE
queues.
"""

import os

import numpy as np
import ml_dtypes

import concourse.bass as bass
import concourse.bacc as bacc_mod
import concourse.hw_specs as hw_specs
import concourse.tile as tile
import concourse.mybir as mybir
from concourse.bass_utils import run_bass_kernel_spmd

BF16 = mybir.dt.bfloat16
F32 = mybir.dt.float32
F32R = mybir.dt.float32r
F8 = mybir.dt.float8e4
I16 = mybir.dt.int16
DR = mybir.MatmulPerfMode.DoubleRow
AF = mybir.ActivationFunctionType
OP = mybir.AluOpType

P = 128
EPS = 1e-5

# full-problem dims
D_MODEL = 1024
D_FF = 4096
N_HEADS = 16
D_K = 64
SEQ = 2048
TQ = 512          # queries per core
N_CORES = 8


def build_program(D=D_MODEL, DFF=D_FF, H=N_HEADS, S=SEQ, T=TQ):
    """Emit the per-core Bass program (SPMD: same NEFF on all cores)."""
    KO = D // P            # feature chunks of d_model
    FO = DFF // P          # feature chunks of d_ff
    TC = S // P            # key-token chunks
    HP = H // 2            # head pairs (even head on partitions 0-63, odd on 64-127)
    VW = 65                # v-aug row width: 64 v cols + ones col
    WS = min(512, D)       # weight-stream chunk width
    SC = min(512, S)       # score/psum free chunk width
    MI = WS // P
    assert H * D_K == D and TC % 4 == 0 and T <= 512

    nc = bacc_mod.Bacc()

    xT_d = nc.dram_tensor("xT", (P, D // 256, 2, S), F8, kind="ExternalInput")
    xTq_d = nc.dram_tensor("xTq", (P, D // 256, 2, T), F8, kind="ExternalInput")
    xres_d = nc.dram_tensor("xres", (D, T), F32, kind="ExternalInput")
    Wq_d = nc.dram_tensor("Wq", (P, D // 256, 2, D), F8, kind="ExternalInput")
    Wk_d = nc.dram_tensor("Wk", (P, D // 256, 2, D), F8, kind="ExternalInput")
    Wv_d = nc.dram_tensor("Wv", (P, D // 256, 2, D), F8, kind="ExternalInput")
    Wo_d = nc.dram_tensor("Wo", (D, D), BF16, kind="ExternalInput")
    # fp8 DoubleRow-interleaved FFN weights, pre-scaled by 32 on the host
    w1q_d = nc.dram_tensor("w1q", (P, D // 256, 2, DFF), F8, kind="ExternalInput")
    # W2 packed per-output-chunk contiguous: [KO][P, DFF//256, 2, P]
    w2q_d = nc.dram_tensor("w2q", (KO, P, DFF // 256, 2, P), F8,
                           kind="ExternalInput")
    # packed per-partition biases: [bq | bk | b1 | b2] as [P, KO+KO+FO+KO]
    cpk_d = nc.dram_tensor("cpk", (P, 3 * KO + FO), F32, kind="ExternalInput")
    # rank-1 LN1-fold rows: [colsum(W1) ; b1] as [2, DFF] bf16
    c1r_d = nc.dram_tensor("c1r", (2, DFF), BF16, kind="ExternalInput")
    # output is written feature-major [D, T]; the host transposes
    out_d = nc.dram_tensor("out", (D, T), F32, kind="ExternalOutput")

    def wr(w):  # [K, M] weight dram -> [P, K//P, M] partition-chunked view
        return w[:, :].rearrange("(o p) m -> p o m", p=P)

    with tile.TileContext(nc) as tc:
        with (
            tc.tile_pool(name="sb", bufs=1) as sb,
            tc.tile_pool(name="ps", bufs=1, space="PSUM") as ps,
        ):
            # ---- phase A: projections; q first so the PE starts early ----
            KO2 = KO // 2
            xTq = sb.tile([P, KO2, 2, T], F8, tag="mid", bufs=2, name="xTq")
            qT = sb.tile([P, KO, T], BF16, tag="mid", bufs=2, name="qT")
            # PE warm-up: HAM releases the clock gate after ~3.4us of activity.
            # Junk matmuls on not-yet-written qT (never read back) span the
            # first input DMAs so the real chains start at full clock.
            wps = ps.tile([P, 2, T], F32, tag="mm", bufs=2, name="wps")
            for i in range(5):
                nc.tensor.matmul(wps[:, 0, :], lhsT=qT[:, 0, 0:P], rhs=qT[:, 0, :],
                                 start=(i == 0), stop=(i == 4))
            xT = sb.tile([P, KO2, 2, S], F8, tag="big", bufs=3, name="xT")
            kT = sb.tile([P, KO, S], BF16, tag="big", bufs=3, name="kT")
            vAug = sb.tile([P, TC, H, VW], BF16, tag="big", bufs=3, name="vAug")
            cpk = sb.tile([P, 3 * KO + FO], F32, name="cpk")
            bq_t, bk_t = cpk[:, 0:KO], cpk[:, KO:2 * KO]
            b2_t = cpk[:, 2 * KO + FO:]
            c1r_t = sb.tile([2, DFF], BF16, name="c1r_t")
            onesf_1p = sb.tile([1, P], F32, name="onesf_1p")
            onesf_bcol = sb.tile([P, 1], F32, name="onesf_bcol")
            ones32_1p = sb.tile([1, P], BF16, name="ones32_1p")
            ones_bcol = sb.tile([P, 1], BF16, name="ones_bcol")
            eps_t = sb.tile([1, 1], F32, name="eps_t")

            # startup DMAs: scalar (ACT) issues NO dmas — a dma_start occupies
            # its engine for the whole transfer and ACT is the exp bottleneck.
            # gpsimd (SWDGE) and sync (HWDGE) split the traffic, ordered by
            # first use.
            w4 = S // 4
            wq0 = sb.tile([P, KO2, 2, WS], F8, tag="wst", bufs=3, name="wq0")
            wq1 = sb.tile([P, KO2, 2, WS], F8, tag="wst", bufs=3, name="wq1")
            nc.gpsimd.dma_start(xTq, xTq_d[:, :, :, :])
            nc.sync.dma_start(wq0, Wq_d[:, :, :, 0:WS])
            nc.gpsimd.dma_start(cpk, cpk_d[:, :])
            nc.sync.dma_start(wq1, Wq_d[:, :, :, WS:2 * WS])
            nc.gpsimd.dma_start(xT[:, :, :, 0:w4], xT_d[:, :, :, 0:w4])
            nc.gpsimd.dma_start(xT[:, :, :, 2 * w4:3 * w4],
                                xT_d[:, :, :, 2 * w4:3 * w4])
            nc.vector.memset(onesf_1p, 1.0)
            nc.vector.memset(onesf_bcol, 1.0)
            nc.vector.memset(ones32_1p, 32.0)
            nc.vector.memset(ones_bcol, 1.0)
            nc.vector.memset(eps_t, EPS)
            # touch ACT immediately so its one-time table load (~2.7us with
            # drain) runs during the initial DMA wait instead of stalling the
            # first Q eviction
            nc.scalar.activation(eps_t, eps_t, AF.Copy, bias=0.0, scale=1.0)

            for mo2 in range(D // WS):
                wt = (wq0, wq1)[mo2]
                for mi in range(0, MI, 2):
                    pst = ps.tile([P, 2, T], F32, tag="mm", bufs=2, name=f"qp{mo2}_{mi}")
                    for half in range(2):
                        mo = mo2 * MI + mi + half
                        msl = slice((mi + half) * P, (mi + half + 1) * P)
                        for o2 in range(KO2):
                            nc.tensor.matmul(
                                pst[:, half, :],
                                lhsT=wt[:, o2, :, msl],
                                rhs=xTq[:, o2, :, :], perf_mode=DR,
                                start=(o2 == 0), stop=(o2 == KO2 - 1))
                        nc.scalar.activation(qT[:, mo, :], pst[:, half, :], AF.Identity,
                                             bias=bq_t[:, mo:mo + 1], scale=1.0 / 32)

            # k^T [D, S]: two token-chunk chains per psum tile, one batched evict
            for mo2 in range(D // WS):
                wt = sb.tile([P, KO2, 2, WS], F8, tag="wst", bufs=3, name=f"wk{mo2}")
                nc.sync.dma_start(wt, Wk_d[:, :, :, mo2 * WS:(mo2 + 1) * WS])
                if mo2 == 0:
                    nc.sync.dma_start(xT[:, :, :, w4:2 * w4],
                                      xT_d[:, :, :, w4:2 * w4])
                    nc.sync.dma_start(xT[:, :, :, 3 * w4:4 * w4],
                                      xT_d[:, :, :, 3 * w4:4 * w4])
                for mi in range(MI):
                    mo = mo2 * MI + mi
                    n_ch = S // SC
                    for nc2 in range((n_ch + 1) // 2):
                        w = min(2, n_ch - nc2 * 2)
                        pst = ps.tile([P, 2, SC], F32, tag="mm", bufs=2,
                                      name=f"kp{mo}_{nc2}")
                        for half in range(w):
                            ncc = nc2 * 2 + half
                            for o2 in range(KO2):
                                nc.tensor.matmul(pst[:, half, :],
                                                 lhsT=wt[:, o2, :, mi * P:(mi + 1) * P],
                                                 rhs=xT[:, o2, :, ncc * SC:(ncc + 1) * SC],
                                                 perf_mode=DR,
                                                 start=(o2 == 0), stop=(o2 == KO2 - 1))
                        nc.scalar.activation(
                            kT[:, mo, nc2 * 2 * SC:nc2 * 2 * SC + w * SC],
                            pst[:, 0:w, :], AF.Identity,
                            bias=bk_t[:, mo:mo + 1], scale=1.0 / 32)

            xres = sb.tile([P, KO, T], F32, tag="res", bufs=2, name="xres")

            # v token-major [S, D] with appended ones column per head:
            # vAug[p, tc, h, 0:64] = v[tc*128+p, h*64:(h+1)*64],  vAug[.., 64] = 1
            # (bv is folded into xres on the host: softmax weights sum to 1)
            # Evictions run on DVE so ACT stays exp-only during attention.
            nc.vector.memset(vAug[:, :, :, D_K:D_K + 1], 1.0)
            wv0 = sb.tile([P, KO2, 2, WS], F8, tag="wst", bufs=3, name="wv0")
            wv1 = sb.tile([P, KO2, 2, WS], F8, tag="wst", bufs=3, name="wv1")
            nc.sync.dma_start(wv0, Wv_d[:, :, :, 0:WS])
            nc.sync.dma_start(wv1, Wv_d[:, :, :, WS:2 * WS])
            # xres (2MB, first needed ~10us into attention) rides the SYNC
            # ring BEHIND all projection weights: engine DMA queues run ahead
            # of program order, so on gpsimd this transfer fired mid-K-phase
            # and starved the weight streams of aggregate SDMA bandwidth
            # (wk0 was observed arriving 17us after issue)
            nc.sync.dma_start(xres, xres_d[:, :].rearrange("(o p) t -> p o t", p=P))
            nh = WS // D_K

            def v_chunk(no2, tc_, wt):
                pfull = ps.tile([P, 2, SC], F32, tag="mm", bufs=2,
                                name=f"vp{no2}_{tc_}")
                pst = pfull[:, 0, :WS]
                for o2 in range(KO2):
                    nc.tensor.matmul(pst,
                                     lhsT=xT[:, o2, :, tc_ * P:(tc_ + 1) * P],
                                     rhs=wt[:, o2, :, :], perf_mode=DR,
                                     start=(o2 == 0), stop=(o2 == KO2 - 1))
                nc.scalar.activation(
                    vAug[:, tc_, no2 * nh:(no2 + 1) * nh, 0:D_K],
                    pst.rearrange("p (h d) -> p h d", d=D_K),
                    AF.Copy, bias=0.0, scale=1.0 / 32)

            for tc_ in range(TC):
                v_chunk(0, tc_, wv0)
            for tc_ in range(TC):
                v_chunk(1, tc_, wv1)

            # W1's 4MB preload is gated on xT's big-ring slot being released
            # by the last V chain, so it cannot contend with the startup
            nc.gpsimd.dma_start(c1r_t, c1r_d[:, :])
            w1q = sb.tile([P, D // 256, 2, DFF], F8, tag="big", bufs=3, name="w1q")
            nc.gpsimd.dma_start(w1q, w1q_d[:, :, :, :])

            def bcast_prep(t):
                # stream_shuffle streams all 32 input lanes; zero the quadrant
                # BEFORE the row-0 write so nothing is read uninitialized.
                nc.vector.memset(t[0:32, :], 0.0)

            def bcast_from_row0(t, rows=128):
                """Replicate t[0:1, :] (SBUF) to partitions 0..rows, DVE-only."""
                nc.vector.stream_shuffle(t[32:64, :], t[0:32, :], mask=[0] * 32)
                nc.vector.tensor_copy(t[0:32, :], t[32:64, :])
                if rows > 64:
                    nc.vector.tensor_copy(t[64:96, :], t[32:64, :])
                    nc.vector.tensor_copy(t[96:128, :], t[32:64, :])

            # ---- phase B: attention with fused out-projection ----
            # Per kc chunk both heads of a pair land in ONE [P, 2, T] psum tile
            # so a single ACT exp covers 1024 elements. ctx row 64 = softmax
            # denominator (ones column of vAug). ctx accumulators live in a
            # 3-deep "cps" ring (pair p+1 never waits on pair p's normalize);
            # pair p's out-projection matmuls interleave into pair p+1's kc
            # loop against a dedicated 1-bank "op" ring, each eviction a DVE
            # add into xres.
            def norm_muls(hp, cpsA, cpsB):
                ctxp = sb.tile([P, T], BF16, tag="ctxp", bufs=2, name=f"cx{hp}")
                for h, cps in ((2 * hp, cpsA), (2 * hp + 1, cpsB)):
                    base = D_K * (h % 2)
                    bcs = sb.tile([P, T], F32, tag="scr", bufs=3, name=f"bc{h}")
                    bcast_prep(bcs)
                    # custom-DVE ops mis-read PSUM: stage the denominator row
                    # into SBUF first, then reciprocal in place.
                    nc.vector.tensor_copy(bcs[0:1, :], cps[D_K:D_K + 1, :])
                    nc.vector.reciprocal_approx_fast(bcs[0:1, :], bcs[0:1, :])
                    bcast_from_row0(bcs, rows=64)
                    nc.vector.tensor_mul(ctxp[base:base + D_K, :],
                                         cps[0:D_K, :], bcs[0:D_K, :])
                return ctxp

            def emit_op(hp, ctxp, wo_t, mo):
                # acc-ring op tile: during pair p's loop the ring holds the
                # two open ctx accumulators plus two free slots (pair p-1's,
                # released by its norm_muls at the top of pair p); the
                # allocator hands out free slots, so these never touch the
                # open accumulators or the score-tile (mm) ring.
                op = ps.tile([P, T], F32, tag="acc", bufs=4, name=f"o{hp}_{mo}")
                nc.tensor.matmul(op, lhsT=wo_t[:, mo * P:(mo + 1) * P],
                                 rhs=ctxp, start=True, stop=True)
                nc.vector.tensor_add(xres[:, mo, :], op, xres[:, mo, :])

            def op_pair(wo_t, ctxp, mg, name):
                """Two out-proj chunks into one mm-ring tile + one DVE add."""
                op2 = ps.tile([P, 2, T], F32, tag="mm", bufs=2, name=name)
                for half in range(2):
                    mo = 2 * mg + half
                    nc.tensor.matmul(op2[:, half, :],
                                     lhsT=wo_t[:, mo * P:(mo + 1) * P],
                                     rhs=ctxp, start=True, stop=True)
                xr2 = xres[:, 2 * mg:2 * mg + 2, :]
                nc.vector.tensor_add(xr2, op2, xr2)
                return xr2

            # Pair p-1's normalize (all-DVE) is emitted at pair p's top and
            # runs under the first ~6 exps; its 8 out-proj matmuls then
            # interleave one-per-2-kc from kc=6 (5 in-loop, 3 at the
            # boundary), filling PE slack in the ACT-bound steady state so
            # only pair 7's normalize+out-proj remains after the last exp.
            pend = None
            opnd = None
            for hp in range(HP):
                hA, hB = 2 * hp, 2 * hp + 1
                wo_t = sb.tile([P, D], BF16, tag="wo", bufs=2, name=f"wo{hp}")
                nc.sync.dma_start(wo_t, wr(Wo_d)[:, hp, :])
                cpsA = ps.tile([P, T], F32, tag="acc", bufs=4, name=f"cA{hp}")
                cpsB = ps.tile([P, T], F32, tag="acc", bufs=4, name=f"cB{hp}")
                if pend is not None:
                    opnd = (pend[0], norm_muls(*pend[:3]), pend[3])
                for kc in range(TC):
                    s2 = ps.tile([P, 2, T], F32, tag="mm", bufs=2, name=f"s{hp}_{kc}")
                    nc.tensor.matmul(s2[:, 0, :],
                                     lhsT=kT[0:D_K, hp, kc * P:(kc + 1) * P],
                                     rhs=qT[0:D_K, hp, :], start=True, stop=True)
                    nc.tensor.matmul(s2[:, 1, :],
                                     lhsT=kT[D_K:P, hp, kc * P:(kc + 1) * P],
                                     rhs=qT[D_K:P, hp, :], start=True, stop=True)
                    e2 = sb.tile([P, 2, T], BF16, tag="e", bufs=4,
                                 name=f"e{hp}_{kc}")
                    nc.scalar.activation(e2, s2, AF.Exp, scale=0.125)
                    nc.tensor.matmul(cpsA[0:D_K + 1, :],
                                     lhsT=vAug[:, kc, hA, 0:D_K + 1],
                                     rhs=e2[:, 0, :],
                                     start=(kc == 0), stop=(kc == TC - 1))
                    nc.tensor.matmul(cpsB[0:D_K + 1, :],
                                     lhsT=vAug[:, kc, hB, 0:D_K + 1],
                                     rhs=e2[:, 1, :],
                                     start=(kc == 0), stop=(kc == TC - 1))
                    if opnd is not None and ((kc >= 6 and kc % 2 == 0)
                                             or kc == 15):
                        j = 5 if kc == 15 else (kc - 6) // 2
                        emit_op(opnd[0], opnd[1], opnd[2], j)
                if opnd is not None:
                    for mo in (6, 7):
                        emit_op(opnd[0], opnd[1], opnd[2], mo)
                    opnd = None
                pend = (hp, cpsA, cpsB, wo_t)

            # ---- pair 7 out-projection with LN1 stats interleaved ----
            # No exp pressure remains: per 2 chunks, two op matmuls into one
            # mm-ring tile, ONE DVE add, ONE 1024-col fp8 quantize, then the
            # LN1 stat chains (LN1 itself is folded into FFN1).
            ctxp7, wo7 = norm_muls(7, cpsA, cpsB), wo_t
            xres_q = sb.tile([P, KO // 2, 2, T], F8, tag="mid", bufs=2, name="xres_q")
            sum1_ps = ps.tile([1, T], F32, tag="acc", bufs=4, name="sum1_ps")
            ssq1_ps = ps.tile([1, T], F32, tag="acc", bufs=4, name="ssq1_ps")
            for mg in range(KO // 2):
                xr2 = op_pair(wo7, ctxp7, mg, f"o7_{mg}")
                xq2 = xres_q[:, mg, :, :]
                nc.scalar.activation(xq2, xr2, AF.Copy, bias=0.0, scale=1.0)
                ysq = sb.tile([P, 2, T], BF16, tag="ysq", bufs=2, name=f"ys1_{mg}")
                nc.vector.tensor_mul(ysq, xq2, xq2)
                for half in range(2):
                    mo = 2 * mg + half
                    nc.tensor.matmul(sum1_ps, lhsT=ones_bcol, rhs=xq2[:, half, :],
                                     start=(mo == 0), stop=(mo == KO - 1))
                    nc.tensor.matmul(ssq1_ps, lhsT=ones_bcol, rhs=ysq[:, half, :],
                                     start=(mo == 0), stop=(mo == KO - 1))

            # ---- phase C: LN1 rows (stats only; LN1 is folded into FFN) ----
            # std via ACT Sqrt (one table switch, stays loaded through LN2),
            # rstd via the standard DVE reciprocal.
            mu1f = sb.tile([1, T], F32, tag="lns", bufs=4, name="mu1f")
            nc.scalar.activation(mu1f, sum1_ps, AF.Copy, bias=0.0, scale=1.0 / D)
            var1 = sb.tile([1, T], F32, tag="lns", bufs=4, name="var1")
            nc.vector.tensor_mul(var1, mu1f, mu1f)
            nc.vector.scalar_tensor_tensor(out=var1, in0=ssq1_ps, scalar=1.0 / D,
                                           in1=var1, op0=OP.mult, op1=OP.subtract)
            std1 = sb.tile([1, T], F32, tag="lns", bufs=4, name="std1")
            nc.scalar.activation(std1, var1, AF.Sqrt, bias=eps_t[0:1, 0:1],
                                 scale=1.0)
            rstd1_sb = sb.tile([P, T], F32, tag="scr", bufs=3, name="rstd1_sb")
            bcast_prep(rstd1_sb)
            nc.vector.reciprocal(rstd1_sb[0:1, :], std1)
            bcast_from_row0(rstd1_sb)
            # pack [-mu ; std] contiguously in one partition-0 stage tile and
            # ship both rows with a single partition-crossing DMA (engine APs
            # can't write partition 1, but DMA descriptors can)
            rstage = sb.tile([1, 2, T], BF16, tag="lns", bufs=4, name="rstage")
            nc.scalar.activation(rstage[0:1, 0, :], sum1_ps, AF.Copy, bias=0.0,
                                 scale=-1.0 / D)
            nc.scalar.activation(rstage[0:1, 1, :], std1, AF.Copy, bias=0.0,
                                 scale=1.0)
            negmu_row = rstage[0:1, 0, :]
            rmix = sb.tile([2, T], BF16, tag="lns", bufs=4, name="rmix")
            # two plain partition-targeted DMAs: the single partition-
            # expanding transfer was observed to race its completion
            # semaphore (stride-4 column groups landing late)
            nc.sync.dma_start(rmix[0:1, :], rstage[0:1, 0, :])
            nc.sync.dma_start(rmix[1:2, :], rstage[0:1, 1, :])

            # ---- phase D: FFN1 + relu (std-scaled domain, no bias on ACT) ----
            # fp8 DoubleRow: 4 K=256 matmuls per chain; psum carries 32x the
            # true value (host-scaled weights), rescaled at the relu eviction.
            rT = sb.tile([P, FO // 2, 2, T], F8, tag="big", bufs=3, name="rT")
            for fo2 in range(DFF // WS):
                for fi in range(0, MI, 2):
                    pst = ps.tile([P, 2, T], F32, tag="mm", bufs=2, name=f"zp{fo2}_{fi}")
                    for half in range(2):
                        fo = fo2 * MI + fi + half
                        for o2 in range(KO // 2):
                            nc.tensor.matmul(pst[:, half, :],
                                             lhsT=w1q[:, o2, :, fo * P:(fo + 1) * P],
                                             rhs=xres_q[:, o2, :, :],
                                             perf_mode=DR,
                                             start=(o2 == 0), stop=False)
                        # K=2 rank-1 fold: (-mu)(x)colsum(W1) + std(x)b1
                        nc.tensor.matmul(pst[:, half, :],
                                         lhsT=c1r_t[0:2, fo * P:(fo + 1) * P],
                                         rhs=rmix[0:2, :], start=False, stop=True)
                    # both halves land in the same rT [., fo//2, 0:2, .]
                    # slice: ONE batched 1024-col relu eviction per psum tile
                    fo = fo2 * MI + fi
                    nc.scalar.activation(rT[:, fo // 2, :, :], pst,
                                         AF.Relu, bias=0.0, scale=1.0 / 32)

            # ---- phase E: FFN2 + residual + LN2 stats (interleaved) ----
            y2 = sb.tile([P, KO, T], F32, tag="res", bufs=2, name="y2")
            sum2_ps = ps.tile([1, T], F32, tag="acc", bufs=4, name="sum2_ps")
            ssq2_ps = ps.tile([1, T], F32, tag="acc", bufs=4, name="ssq2_ps")
            for mo in range(KO):
                pfull = ps.tile([P, 2, T], F32, tag="mm", bufs=2, name=f"fp{mo}")
                pst = pfull[:, 0, :]
                w2t = sb.tile([P, DFF // 256, 2, P], F8, tag="w2", bufs=2,
                              name=f"w2_{mo}")
                nc.sync.dma_start(w2t, w2q_d[mo])
                for ki in range(DFF // 256):
                    nc.tensor.matmul(pst, lhsT=w2t[:, ki, :, :],
                                     rhs=rT[:, ki, :, :], perf_mode=DR,
                                     start=(ki == 0), stop=False)
                # rank-1: subtract 32*mu1 (broadcast over features) in-psum
                nc.tensor.matmul(pst, lhsT=ones32_1p[0:1, :],
                                 rhs=negmu_row, start=False, stop=True)
                # y2 = rstd1*(ffpsum/32 + xres - mu1) + b2
                nc.vector.scalar_tensor_tensor(out=y2[:, mo, :], in0=pst,
                                               scalar=1.0 / 32, in1=xres[:, mo, :],
                                               op0=OP.mult, op1=OP.add)
                nc.vector.tensor_mul(y2[:, mo, :], y2[:, mo, :], rstd1_sb)
                nc.vector.tensor_scalar_add(y2[:, mo, :], y2[:, mo, :],
                                            b2_t[:, mo:mo + 1])
                # LN2 stats accumulate as chunks complete; the sum chain
                # consumes y2 (f32) directly — no bf16 staging copy
                ysq2 = sb.tile([P, T], BF16, tag="ysq", bufs=2, name=f"ys2_{mo}")
                nc.scalar.activation(ysq2, y2[:, mo, :], AF.Square, bias=0.0,
                                     scale=1.0)
                nc.tensor.matmul(sum2_ps, lhsT=onesf_bcol, rhs=y2[:, mo, :],
                                 start=(mo == 0), stop=(mo == KO - 1))
                nc.tensor.matmul(ssq2_ps, lhsT=ones_bcol, rhs=ysq2,
                                 start=(mo == 0), stop=(mo == KO - 1))

            # ---- phase F: LN2 normalize feature-major + store ----
            # -mu2 and rstd2 rows are PE-broadcast to all 128 partitions via
            # rank-1 matmuls into PSUM; the normalize is then two DVE
            # tensor_tensor ops per chunk and the output DMAs feature-major
            # (the host transposes).
            negmu2 = sb.tile([1, T], F32, tag="lns", bufs=4, name="negmu2")
            nc.scalar.activation(negmu2, sum2_ps, AF.Copy, bias=0.0,
                                 scale=-1.0 / D)
            t2m = sb.tile([1, T], F32, tag="lns", bufs=4, name="t2m")
            nc.vector.tensor_mul(t2m, negmu2, negmu2)
            var2 = sb.tile([1, T], F32, tag="lns", bufs=4, name="var2")
            nc.vector.scalar_tensor_tensor(out=var2, in0=ssq2_ps, scalar=1.0 / D,
                                           in1=t2m, op0=OP.mult, op1=OP.subtract)
            std2 = sb.tile([1, T], F32, tag="lns", bufs=4, name="std2")
            nc.scalar.activation(std2, var2, AF.Sqrt, bias=eps_t[0:1, 0:1],
                                 scale=1.0)
            rstd2 = sb.tile([1, T], F32, tag="lns", bufs=4, name="rstd2")
            nc.vector.reciprocal(rstd2, std2)
            nm2_ps = ps.tile([P, T], F32, tag="acc", bufs=4, name="nm2_ps")
            rs2_ps = ps.tile([P, T], F32, tag="acc", bufs=4, name="rs2_ps")
            nc.tensor.matmul(nm2_ps, lhsT=onesf_1p, rhs=negmu2,
                             start=True, stop=True)
            nc.tensor.matmul(rs2_ps, lhsT=onesf_1p, rhs=rstd2,
                             start=True, stop=True)
            out_r = out_d[:, :].rearrange("(o p) t -> p o t", p=P)
            for mo in range(KO):
                y2n = sb.tile([P, T], F32, tag="scr", bufs=3, name=f"y2n_{mo}")
                nc.vector.tensor_add(y2n, y2[:, mo, :], nm2_ps)
                nc.vector.tensor_mul(y2n, y2n, rs2_ps)
                # scalar (ACT) is idle at the tail; both HWDGE queues share
                # the output so the last chunk lands as early as possible
                (nc.sync, nc.scalar)[mo % 2].dma_start(out_r[:, mo, :], y2n)

    nc.finalize()
    return nc


def _maybe_enable_ldw_opt():
    if os.environ.get("BASS_LDW_OPT") != "1":
        return
    import concourse.bass_utils as _bu
    if getattr(_bu, "_ldw_opt_patched", False):
        return
    _orig = _bu.run_command

    def _patched(argv, **kw):
        argv = ["--enable-ldw-opt=true" if a == "--enable-ldw-opt=false" else a
                for a in argv]
        return _orig(argv, **kw)

    _bu.run_command = _patched
    _bu._ldw_opt_patched = True


_maybe_enable_ldw_opt()

_PROG = None
_last_results = None


def _get_prog():
    global _PROG
    if _PROG is None:
        _PROG = build_program()
    return _PROG


def pack_consts(bq, bk, b1, b2, KO=D_MODEL // P, FO=D_FF // P):
    cols = []
    for vec, n in ((bq, KO), (bk, KO), (b1, FO), (b2, KO)):
        cols.append(np.asarray(vec, np.float32).reshape(n, P).T)  # [P, n]
    return np.ascontiguousarray(np.concatenate(cols, axis=1))


def make_in_maps(x, Wq, bq, Wk, bk, Wv, bv, Wo, bo, W1, b1, W2, b2,
                 ln1_g, ln1_b, ln2_g, ln2_b):
    bf = ml_dtypes.bfloat16
    f32 = np.float32
    x = np.asarray(x, f32)
    f8 = ml_dtypes.float8_e4m3
    W1f = np.asarray(W1, f32)
    c1r = (32.0 * np.stack([W1f.sum(axis=0), np.asarray(b1, f32)])).astype(bf)

    def pack_dr(w):  # [K, M] -> [P, K//256, 2, M] fp8, pre-scaled by 32
        K, M = w.shape
        wi = (np.asarray(w, f32) * 32.0).reshape(K // 256, 2, P, M)
        return np.ascontiguousarray(wi.transpose(2, 0, 1, 3).astype(f8))

    w2q = pack_dr(np.asarray(W2, f32))          # [P, DFF//256, 2, D]
    # repack per-output-chunk contiguous: [KO, P, DFF//256, 2, P]
    w2q = np.ascontiguousarray(
        w2q.reshape(P, D_FF // 256, 2, D_MODEL // P, P).transpose(3, 0, 1, 2, 4))

    shared = {
        "Wq": pack_dr(np.asarray(Wq, f32)),
        "Wk": pack_dr(np.asarray(Wk, f32)),
        "Wv": pack_dr(np.asarray(Wv, f32)),
        "Wo": np.ascontiguousarray(np.asarray(Wo, f32).astype(bf)),
        "w1q": pack_dr(W1f),
        "w2q": w2q,
        "cpk": pack_consts(bq, bk, b1, b2),
        "c1r": np.ascontiguousarray(c1r),
    }
    # bv is invariant under softmax averaging: attn(v + bv) = attn(v) + bv,
    # so fold bv@Wo + bo into the residual once on the host (exact, f32).
    res_bias = (np.asarray(bv, f32) @ np.asarray(Wo, f32)
                + np.asarray(bo, f32))

    def pack_act(a):  # [D, Ntok] -> [P, D//256, 2, Ntok] fp8 interleaved
        Dd, Nt = a.shape
        return np.ascontiguousarray(
            a.reshape(Dd // 256, 2, P, Nt).transpose(2, 0, 1, 3).astype(f8))

    in_maps = []
    xT_by_batch = [np.ascontiguousarray(x[b].T) for b in range(x.shape[0])]
    xTq_by_batch = [pack_act(t) for t in xT_by_batch]
    for c in range(N_CORES):
        b, q0 = c // 4, (c % 4) * TQ
        xslice = xT_by_batch[b][:, q0:q0 + TQ]
        m = dict(shared)
        m["xT"] = xTq_by_batch[b]
        m["xTq"] = np.ascontiguousarray(xTq_by_batch[b][:, :, :, q0:q0 + TQ])
        m["xres"] = np.ascontiguousarray(xslice + res_bias[:, None])
        in_maps.append(m)
    return in_maps


def kernel(**inputs):
    global _last_results
    nc = _get_prog()
    in_maps = make_in_maps(**inputs)
    res = run_bass_kernel_spmd(nc, in_maps, core_ids=list(range(N_CORES)),
                               tmpdir=os.environ.get("BASS_KERNEL_TMPDIR"))
    _last_results = res
    x = np.asarray(inputs["x"])
    B, S, D = x.shape
    out = np.empty((B, S, D), np.float32)
    for c in range(N_CORES):
        b, q0 = c // 4, (c % 4) * TQ
        out[b, q0:q0 + TQ, :] = res.results[c]["out"].T
    return out


# revision 62
# speedup vs baseline: 1.1945x; 1.0044x over previous
"""Trainium2 Bass kernel for a dense transformer encoder layer.

Model (fp32 reference):
    q,k,v = x@Wq+bq, x@Wk+bk, x@Wv+bv          (16 heads, d_k=64)
    attn  = softmax(q k^T / 8) v
    h     = LN(x + attn@Wo + bo)
    out   = LN(h + relu(h@W1+b1)@W2 + b2)      (ln gamma=1, beta=0)

Sharding: query-parallel over 8 cores. Core c handles batch b=c//4,
query rows (c%4)*512..+512. Each core recomputes K/V for its batch's
full 2048-token sequence (no collectives needed); host concatenates the
8 [512, 1024] output slices (the device writes feature-major; the host
transposes).

On-device layout: activations feature-major ([feature, token]) end to
end; scores transposed ([k_tok, q]) so softmax denominators come free
from a ones-column appended to V.

Precision: all five projection groups run as fp8-e4m3 DoubleRow
matmuls (weights host-prescaled by 32, 1/32 folded into evictions).
Scores/ctx stay bf16. bv is exact under softmax averaging, so V is
projected bias-free and bv@Wo+bo is folded into xres on the host.

Engine discipline learned from traces: a dma_start OCCUPIES its
issuing engine for the whole transfer, so the scalar (ACT) engine —
the attention-exp bottleneck — issues no DMAs before the output tail;
sync (HWDGE) and gpsimd (SWDGE) split all traffic as whole-tensor
transfers ordered by first use. W2 is host-repacked per-output-chunk
contiguous and streamed as one 512KB sync DMA per chunk. Attention
emits pair p's normalize+out-projection after pair p+1's score/ctx
matmuls (the PE stream is static, so this keeps the exp pipeline
dense while the all-DVE normalize drains); the attention exp stream
runs within ~6% of the 134us ScalarE floor.

LayerNorm 1 is never materialized: FFN1 consumes fp8(xres) with a K=2
rank-1 fold per chain adding (-mu)(x)colsum(W1) + std(x)b1; relu runs
in the std-scaled domain and rstd1 multiplies at the FFN2 eviction
(which also carries a rank-1 -mu1 fold); both halves of each FFN1
psum tile evict through ONE batched 1024-col relu, halving the
ACT-eviction handshakes. LN1/LN2 std comes from ACT
Sqrt (one table switch, in the post-attention trough) and rstd from
the DVE reciprocal; the fold rows ship as two plain partition-
targeted DMAs (a single partition-expanding transfer raced its
completion semaphore). LN2 statistics accumulate inside the FFN2
loop (the sum chain reads y2 f32 directly); -mu2/rstd2 broadcast via
rank-1 PE matmuls into PSUM and the normalize is two DVE ops per
feature-major chunk with per-chunk output DMAs on both idle HWDGE
queues.
"""

import os

import numpy as np
import ml_dtypes

import concourse.bass as bass
import concourse.bacc as bacc_mod
import concourse.hw_specs as hw_specs
import concourse.tile as tile
import concourse.mybir as mybir
from concourse.bass_utils import run_bass_kernel_spmd

BF16 = mybir.dt.bfloat16
F32 = mybir.dt.float32
F32R = mybir.dt.float32r
F8 = mybir.dt.float8e4
I16 = mybir.dt.int16
DR = mybir.MatmulPerfMode.DoubleRow
AF = mybir.ActivationFunctionType
OP = mybir.AluOpType

P = 128
EPS = 1e-5

# full-problem dims
D_MODEL = 1024
D_FF = 4096
N_HEADS = 16
D_K = 64
SEQ = 2048
TQ = 512          # queries per core
N_CORES = 8


def build_program(D=D_MODEL, DFF=D_FF, H=N_HEADS, S=SEQ, T=TQ):
    """Emit the per-core Bass program (SPMD: same NEFF on all cores)."""
    KO = D // P            # feature chunks of d_model
    FO = DFF // P          # feature chunks of d_ff
    TC = S // P            # key-token chunks
    HP = H // 2            # head pairs (even head on partitions 0-63, odd on 64-127)
    VW = 65                # v-aug row width: 64 v cols + ones col
    WS = min(512, D)       # weight-stream chunk width
    SC = min(512, S)       # score/psum free chunk width
    MI = WS // P
    assert H * D_K == D and TC % 4 == 0 and T <= 512

    nc = bacc_mod.Bacc()

    xT_d = nc.dram_tensor("xT", (P, D // 256, 2, S), F8, kind="ExternalInput")
    xTq_d = nc.dram_tensor("xTq", (P, D // 256, 2, T), F8, kind="ExternalInput")
    xres_d = nc.dram_tensor("xres", (D, T), F32, kind="ExternalInput")
    Wq_d = nc.dram_tensor("Wq", (P, D // 256, 2, D), F8, kind="ExternalInput")
    Wk_d = nc.dram_tensor("Wk", (P, D // 256, 2, D), F8, kind="ExternalInput")
    Wv_d = nc.dram_tensor("Wv", (P, D // 256, 2, D), F8, kind="ExternalInput")
    Wo_d = nc.dram_tensor("Wo", (D, D), BF16, kind="ExternalInput")
    # fp8 DoubleRow-interleaved FFN weights, pre-scaled by 32 on the host
    # W1 packed per-output-chunk contiguous ([P][fo][K//256][2][P]) so the
    # DR LDWEIGHTS source slices are small-stride like W2's
    w1q_d = nc.dram_tensor("w1q", (P, DFF // P, D // 256, 2, P), F8,
                           kind="ExternalInput")
    # W2 packed per-output-chunk contiguous: [KO][P, DFF//256, 2, P]
    w2q_d = nc.dram_tensor("w2q", (KO, P, DFF // 256, 2, P), F8,
                           kind="ExternalInput")
    # packed per-partition biases: [bq | bk | b1 | b2] as [P, KO+KO+FO+KO]
    cpk_d = nc.dram_tensor("cpk", (P, 3 * KO + FO), F32, kind="ExternalInput")
    # rank-1 LN1-fold rows: [colsum(W1) ; b1] as [2, DFF] bf16
    c1r_d = nc.dram_tensor("c1r", (2, DFF), BF16, kind="ExternalInput")
    # output is written feature-major [D, T]; the host transposes
    out_d = nc.dram_tensor("out", (D, T), F32, kind="ExternalOutput")

    def wr(w):  # [K, M] weight dram -> [P, K//P, M] partition-chunked view
        return w[:, :].rearrange("(o p) m -> p o m", p=P)

    with tile.TileContext(nc) as tc:
        with (
            tc.tile_pool(name="sb", bufs=1) as sb,
            tc.tile_pool(name="ps", bufs=1, space="PSUM") as ps,
        ):
            # ---- phase A: projections; q first so the PE starts early ----
            KO2 = KO // 2
            xTq = sb.tile([P, KO2, 2, T], F8, tag="mid", bufs=2, name="xTq")
            qT = sb.tile([P, KO, T], BF16, tag="mid", bufs=2, name="qT")
            # PE warm-up: HAM releases the clock gate after ~3.4us of activity.
            # Junk matmuls on not-yet-written qT (never read back) span the
            # first input DMAs so the real chains start at full clock.
            wps = ps.tile([P, 2, T], F32, tag="mm", bufs=2, name="wps")
            for i in range(5):
                nc.tensor.matmul(wps[:, 0, :], lhsT=qT[:, 0, 0:P], rhs=qT[:, 0, :],
                                 start=(i == 0), stop=(i == 4))
            xT = sb.tile([P, KO2, 2, S], F8, tag="big", bufs=3, name="xT")
            kT = sb.tile([P, KO, S], BF16, tag="big", bufs=3, name="kT")
            vAug = sb.tile([P, TC, H, VW], BF16, tag="big", bufs=3, name="vAug")
            cpk = sb.tile([P, 3 * KO + FO], F32, name="cpk")
            bq_t, bk_t = cpk[:, 0:KO], cpk[:, KO:2 * KO]
            b2_t = cpk[:, 2 * KO + FO:]
            c1r_t = sb.tile([2, DFF], BF16, name="c1r_t")
            onesf_1p = sb.tile([1, P], F32, name="onesf_1p")
            onesf_bcol = sb.tile([P, 1], F32, name="onesf_bcol")
            ones32_1p = sb.tile([1, P], BF16, name="ones32_1p")
            ones_bcol = sb.tile([P, 1], BF16, name="ones_bcol")
            eps_t = sb.tile([1, 1], F32, name="eps_t")

            # startup DMAs: scalar (ACT) issues NO dmas — a dma_start occupies
            # its engine for the whole transfer and ACT is the exp bottleneck.
            # gpsimd (SWDGE) and sync (HWDGE) split the traffic, ordered by
            # first use.
            w4 = S // 4
            wq0 = sb.tile([P, KO2, 2, WS], F8, tag="wst", bufs=3, name="wq0")
            wq1 = sb.tile([P, KO2, 2, WS], F8, tag="wst", bufs=3, name="wq1")
            nc.gpsimd.dma_start(xTq, xTq_d[:, :, :, :])
            nc.sync.dma_start(wq0, Wq_d[:, :, :, 0:WS])
            nc.gpsimd.dma_start(cpk, cpk_d[:, :])
            nc.sync.dma_start(wq1, Wq_d[:, :, :, WS:2 * WS])
            nc.gpsimd.dma_start(xT[:, :, :, 0:w4], xT_d[:, :, :, 0:w4])
            nc.gpsimd.dma_start(xT[:, :, :, 2 * w4:3 * w4],
                                xT_d[:, :, :, 2 * w4:3 * w4])
            nc.vector.memset(onesf_1p, 1.0)
            nc.vector.memset(onesf_bcol, 1.0)
            nc.vector.memset(ones32_1p, 32.0)
            nc.vector.memset(ones_bcol, 1.0)
            nc.vector.memset(eps_t, EPS)
            # touch ACT immediately so its one-time table load (~2.7us with
            # drain) runs during the initial DMA wait instead of stalling the
            # first Q eviction
            nc.scalar.activation(eps_t, eps_t, AF.Copy, bias=0.0, scale=1.0)

            for mo2 in range(D // WS):
                wt = (wq0, wq1)[mo2]
                for mi in range(0, MI, 2):
                    pst = ps.tile([P, 2, T], F32, tag="mm", bufs=2, name=f"qp{mo2}_{mi}")
                    for half in range(2):
                        mo = mo2 * MI + mi + half
                        msl = slice((mi + half) * P, (mi + half + 1) * P)
                        for o2 in range(KO2):
                            nc.tensor.matmul(
                                pst[:, half, :],
                                lhsT=wt[:, o2, :, msl],
                                rhs=xTq[:, o2, :, :], perf_mode=DR,
                                start=(o2 == 0), stop=(o2 == KO2 - 1))
                        nc.scalar.activation(qT[:, mo, :], pst[:, half, :], AF.Identity,
                                             bias=bq_t[:, mo:mo + 1], scale=1.0 / 32)

            # k^T [D, S]: two token-chunk chains per psum tile, one batched evict
            for mo2 in range(D // WS):
                wt = sb.tile([P, KO2, 2, WS], F8, tag="wst", bufs=3, name=f"wk{mo2}")
                nc.sync.dma_start(wt, Wk_d[:, :, :, mo2 * WS:(mo2 + 1) * WS])
                if mo2 == 0:
                    nc.sync.dma_start(xT[:, :, :, w4:2 * w4],
                                      xT_d[:, :, :, w4:2 * w4])
                    nc.sync.dma_start(xT[:, :, :, 3 * w4:4 * w4],
                                      xT_d[:, :, :, 3 * w4:4 * w4])
                for mi in range(MI):
                    mo = mo2 * MI + mi
                    n_ch = S // SC
                    for nc2 in range((n_ch + 1) // 2):
                        w = min(2, n_ch - nc2 * 2)
                        pst = ps.tile([P, 2, SC], F32, tag="mm", bufs=2,
                                      name=f"kp{mo}_{nc2}")
                        for half in range(w):
                            ncc = nc2 * 2 + half
                            for o2 in range(KO2):
                                nc.tensor.matmul(pst[:, half, :],
                                                 lhsT=wt[:, o2, :, mi * P:(mi + 1) * P],
                                                 rhs=xT[:, o2, :, ncc * SC:(ncc + 1) * SC],
                                                 perf_mode=DR,
                                                 start=(o2 == 0), stop=(o2 == KO2 - 1))
                        nc.scalar.activation(
                            kT[:, mo, nc2 * 2 * SC:nc2 * 2 * SC + w * SC],
                            pst[:, 0:w, :], AF.Identity,
                            bias=bk_t[:, mo:mo + 1], scale=1.0 / 32)

            xres = sb.tile([P, KO, T], F32, tag="res", bufs=2, name="xres")

            # v token-major [S, D] with appended ones column per head:
            # vAug[p, tc, h, 0:64] = v[tc*128+p, h*64:(h+1)*64],  vAug[.., 64] = 1
            # (bv is folded into xres on the host: softmax weights sum to 1)
            # Evictions run on DVE so ACT stays exp-only during attention.
            nc.vector.memset(vAug[:, :, :, D_K:D_K + 1], 1.0)
            wv0 = sb.tile([P, KO2, 2, WS], F8, tag="wst", bufs=3, name="wv0")
            wv1 = sb.tile([P, KO2, 2, WS], F8, tag="wst", bufs=3, name="wv1")
            nc.sync.dma_start(wv0, Wv_d[:, :, :, 0:WS])
            nc.sync.dma_start(wv1, Wv_d[:, :, :, WS:2 * WS])
            # xres (2MB, first needed ~10us into attention) rides the SYNC
            # ring BEHIND all projection weights: engine DMA queues run ahead
            # of program order, so on gpsimd this transfer fired mid-K-phase
            # and starved the weight streams of aggregate SDMA bandwidth
            # (wk0 was observed arriving 17us after issue)
            nc.sync.dma_start(xres, xres_d[:, :].rearrange("(o p) t -> p o t", p=P))
            nh = WS // D_K

            def v_chunk(no2, tc_, wt):
                pfull = ps.tile([P, 2, SC], F32, tag="mm", bufs=2,
                                name=f"vp{no2}_{tc_}")
                pst = pfull[:, 0, :WS]
                for o2 in range(KO2):
                    nc.tensor.matmul(pst,
                                     lhsT=xT[:, o2, :, tc_ * P:(tc_ + 1) * P],
                                     rhs=wt[:, o2, :, :], perf_mode=DR,
                                     start=(o2 == 0), stop=(o2 == KO2 - 1))
                nc.scalar.activation(
                    vAug[:, tc_, no2 * nh:(no2 + 1) * nh, 0:D_K],
                    pst.rearrange("p (h d) -> p h d", d=D_K),
                    AF.Copy, bias=0.0, scale=1.0 / 32)

            for tc_ in range(TC):
                v_chunk(0, tc_, wv0)
            for tc_ in range(TC):
                v_chunk(1, tc_, wv1)

            # W1's 4MB preload is gated on xT's big-ring slot being released
            # by the last V chain, so it cannot contend with the startup
            nc.gpsimd.dma_start(c1r_t, c1r_d[:, :])
            w1q = sb.tile([P, DFF // P, D // 256, 2, P], F8, tag="big", bufs=3,
                          name="w1q")
            nc.gpsimd.dma_start(w1q, w1q_d[:, :, :, :])

            def bcast_prep(t):
                # stream_shuffle streams all 32 input lanes; zero the quadrant
                # BEFORE the row-0 write so nothing is read uninitialized.
                nc.vector.memset(t[0:32, :], 0.0)

            def bcast_from_row0(t, rows=128):
                """Replicate t[0:1, :] (SBUF) to partitions 0..rows, DVE-only."""
                nc.vector.stream_shuffle(t[32:64, :], t[0:32, :], mask=[0] * 32)
                nc.vector.tensor_copy(t[0:32, :], t[32:64, :])
                if rows > 64:
                    nc.vector.tensor_copy(t[64:96, :], t[32:64, :])
                    nc.vector.tensor_copy(t[96:128, :], t[32:64, :])

            # ---- phase B: attention with fused out-projection ----
            # Per kc chunk both heads of a pair land in ONE [P, 2, T] psum tile
            # so a single ACT exp covers 1024 elements. ctx row 64 = softmax
            # denominator (ones column of vAug). ctx accumulators live in a
            # 3-deep "cps" ring (pair p+1 never waits on pair p's normalize);
            # pair p's out-projection matmuls interleave into pair p+1's kc
            # loop against a dedicated 1-bank "op" ring, each eviction a DVE
            # add into xres.
            def norm_muls(hp, cpsA, cpsB):
                ctxp = sb.tile([P, T], BF16, tag="ctxp", bufs=2, name=f"cx{hp}")
                for h, cps in ((2 * hp, cpsA), (2 * hp + 1, cpsB)):
                    base = D_K * (h % 2)
                    bcs = sb.tile([P, T], F32, tag="scr", bufs=3, name=f"bc{h}")
                    bcast_prep(bcs)
                    # custom-DVE ops mis-read PSUM: stage the denominator row
                    # into SBUF first, then reciprocal in place.
                    nc.vector.tensor_copy(bcs[0:1, :], cps[D_K:D_K + 1, :])
                    nc.vector.reciprocal_approx_fast(bcs[0:1, :], bcs[0:1, :])
                    bcast_from_row0(bcs, rows=64)
                    nc.vector.tensor_mul(ctxp[base:base + D_K, :],
                                         cps[0:D_K, :], bcs[0:D_K, :])
                return ctxp

            def emit_op(hp, ctxp, wo_t, mo):
                # acc-ring op tile: during pair p's loop the ring holds the
                # two open ctx accumulators plus two free slots (pair p-1's,
                # released by its norm_muls at the top of pair p); the
                # allocator hands out free slots, so these never touch the
                # open accumulators or the score-tile (mm) ring.
                op = ps.tile([P, T], F32, tag="acc", bufs=4, name=f"o{hp}_{mo}")
                nc.tensor.matmul(op, lhsT=wo_t[:, mo * P:(mo + 1) * P],
                                 rhs=ctxp, start=True, stop=True)
                nc.vector.tensor_add(xres[:, mo, :], op, xres[:, mo, :])

            def op_pair(wo_t, ctxp, mg, name):
                """Two out-proj chunks into one mm-ring tile + one DVE add."""
                op2 = ps.tile([P, 2, T], F32, tag="mm", bufs=2, name=name)
                for half in range(2):
                    mo = 2 * mg + half
                    nc.tensor.matmul(op2[:, half, :],
                                     lhsT=wo_t[:, mo * P:(mo + 1) * P],
                                     rhs=ctxp, start=True, stop=True)
                xr2 = xres[:, 2 * mg:2 * mg + 2, :]
                nc.vector.tensor_add(xr2, op2, xr2)
                return xr2

            # Pair p-1's normalize (all-DVE) is emitted at pair p's top and
            # runs under the first ~6 exps; its 8 out-proj matmuls then
            # interleave one-per-2-kc from kc=6 (5 in-loop, 3 at the
            # boundary), filling PE slack in the ACT-bound steady state so
            # only pair 7's normalize+out-proj remains after the last exp.
            pend = None
            opnd = None
            for hp in range(HP):
                hA, hB = 2 * hp, 2 * hp + 1
                wo_t = sb.tile([P, D], BF16, tag="wo", bufs=2, name=f"wo{hp}")
                nc.sync.dma_start(wo_t, wr(Wo_d)[:, hp, :])
                cpsA = ps.tile([P, T], F32, tag="acc", bufs=4, name=f"cA{hp}")
                cpsB = ps.tile([P, T], F32, tag="acc", bufs=4, name=f"cB{hp}")
                if pend is not None:
                    opnd = (pend[0], norm_muls(*pend[:3]), pend[3])
                for kc in range(TC):
                    s2 = ps.tile([P, 2, T], F32, tag="mm", bufs=2, name=f"s{hp}_{kc}")
                    nc.tensor.matmul(s2[:, 0, :],
                                     lhsT=kT[0:D_K, hp, kc * P:(kc + 1) * P],
                                     rhs=qT[0:D_K, hp, :], start=True, stop=True)
                    nc.tensor.matmul(s2[:, 1, :],
                                     lhsT=kT[D_K:P, hp, kc * P:(kc + 1) * P],
                                     rhs=qT[D_K:P, hp, :], start=True, stop=True)
                    e2 = sb.tile([P, 2, T], BF16, tag="e", bufs=4,
                                 name=f"e{hp}_{kc}")
                    nc.scalar.activation(e2, s2, AF.Exp, scale=0.125)
                    nc.tensor.matmul(cpsA[0:D_K + 1, :],
                                     lhsT=vAug[:, kc, hA, 0:D_K + 1],
                                     rhs=e2[:, 0, :],
                                     start=(kc == 0), stop=(kc == TC - 1))
                    nc.tensor.matmul(cpsB[0:D_K + 1, :],
                                     lhsT=vAug[:, kc, hB, 0:D_K + 1],
                                     rhs=e2[:, 1, :],
                                     start=(kc == 0), stop=(kc == TC - 1))
                    if opnd is not None and ((kc >= 6 and kc % 2 == 0)
                                             or kc == 15):
                        j = 5 if kc == 15 else (kc - 6) // 2
                        emit_op(opnd[0], opnd[1], opnd[2], j)
                if opnd is not None:
                    for mo in (6, 7):
                        emit_op(opnd[0], opnd[1], opnd[2], mo)
                    opnd = None
                pend = (hp, cpsA, cpsB, wo_t)

            # ---- pair 7 out-projection with LN1 stats interleaved ----
            # No exp pressure remains: per 2 chunks, two op matmuls into one
            # mm-ring tile, ONE DVE add, ONE 1024-col fp8 quantize, then the
            # LN1 stat chains (LN1 itself is folded into FFN1).
            ctxp7, wo7 = norm_muls(7, cpsA, cpsB), wo_t
            xres_q = sb.tile([P, KO // 2, 2, T], F8, tag="mid", bufs=2, name="xres_q")
            sum1_ps = ps.tile([1, T], F32, tag="acc", bufs=4, name="sum1_ps")
            ssq1_ps = ps.tile([1, T], F32, tag="acc", bufs=4, name="ssq1_ps")
            for mg in range(KO // 2):
                xr2 = op_pair(wo7, ctxp7, mg, f"o7_{mg}")
                xq2 = xres_q[:, mg, :, :]
                nc.scalar.activation(xq2, xr2, AF.Copy, bias=0.0, scale=1.0)
                ysq = sb.tile([P, 2, T], BF16, tag="ysq", bufs=2, name=f"ys1_{mg}")
                nc.vector.tensor_mul(ysq, xq2, xq2)
                for half in range(2):
                    mo = 2 * mg + half
                    nc.tensor.matmul(sum1_ps, lhsT=ones_bcol, rhs=xq2[:, half, :],
                                     start=(mo == 0), stop=(mo == KO - 1))
                    nc.tensor.matmul(ssq1_ps, lhsT=ones_bcol, rhs=ysq[:, half, :],
                                     start=(mo == 0), stop=(mo == KO - 1))

            # ---- phase C: LN1 rows (stats only; LN1 is folded into FFN) ----
            # std via ACT Sqrt (one table switch, stays loaded through LN2),
            # rstd via the standard DVE reciprocal.
            mu1f = sb.tile([1, T], F32, tag="lns", bufs=4, name="mu1f")
            nc.scalar.activation(mu1f, sum1_ps, AF.Copy, bias=0.0, scale=1.0 / D)
            var1 = sb.tile([1, T], F32, tag="lns", bufs=4, name="var1")
            nc.vector.tensor_mul(var1, mu1f, mu1f)
            nc.vector.scalar_tensor_tensor(out=var1, in0=ssq1_ps, scalar=1.0 / D,
                                           in1=var1, op0=OP.mult, op1=OP.subtract)
            std1 = sb.tile([1, T], F32, tag="lns", bufs=4, name="std1")
            nc.scalar.activation(std1, var1, AF.Sqrt, bias=eps_t[0:1, 0:1],
                                 scale=1.0)
            rstd1_sb = sb.tile([P, T], F32, tag="scr", bufs=3, name="rstd1_sb")
            bcast_prep(rstd1_sb)
            nc.vector.reciprocal(rstd1_sb[0:1, :], std1)
            bcast_from_row0(rstd1_sb)
            # pack [-mu ; std] contiguously in one partition-0 stage tile and
            # ship both rows with a single partition-crossing DMA (engine APs
            # can't write partition 1, but DMA descriptors can)
            rstage = sb.tile([1, 2, T], BF16, tag="lns", bufs=4, name="rstage")
            nc.scalar.activation(rstage[0:1, 0, :], sum1_ps, AF.Copy, bias=0.0,
                                 scale=-1.0 / D)
            nc.scalar.activation(rstage[0:1, 1, :], std1, AF.Copy, bias=0.0,
                                 scale=1.0)
            negmu_row = rstage[0:1, 0, :]
            rmix = sb.tile([2, T], BF16, tag="lns", bufs=4, name="rmix")
            # two plain partition-targeted DMAs: the single partition-
            # expanding transfer was observed to race its completion
            # semaphore (stride-4 column groups landing late)
            nc.sync.dma_start(rmix[0:1, :], rstage[0:1, 0, :])
            nc.sync.dma_start(rmix[1:2, :], rstage[0:1, 1, :])

            # ---- phase D: FFN1 + relu (std-scaled domain, no bias on ACT) ----
            # fp8 DoubleRow: 4 K=256 matmuls per chain; psum carries 32x the
            # true value (host-scaled weights), rescaled at the relu eviction.
            rT = sb.tile([P, FO // 2, 2, T], F8, tag="big", bufs=3, name="rT")
            for fo2 in range(DFF // WS):
                for fi in range(0, MI, 2):
                    pst = ps.tile([P, 2, T], F32, tag="mm", bufs=2, name=f"zp{fo2}_{fi}")
                    for half in range(2):
                        fo = fo2 * MI + fi + half
                        for o2 in range(KO // 2):
                            nc.tensor.matmul(pst[:, half, :],
                                             lhsT=w1q[:, fo, o2, :, :],
                                             rhs=xres_q[:, o2, :, :],
                                             perf_mode=DR,
                                             start=(o2 == 0), stop=False)
                        # K=2 rank-1 fold: (-mu)(x)colsum(W1) + std(x)b1
                        nc.tensor.matmul(pst[:, half, :],
                                         lhsT=c1r_t[0:2, fo * P:(fo + 1) * P],
                                         rhs=rmix[0:2, :], start=False, stop=True)
                    # both halves land in the same rT [., fo//2, 0:2, .]
                    # slice: ONE batched 1024-col relu eviction per psum tile
                    fo = fo2 * MI + fi
                    nc.scalar.activation(rT[:, fo // 2, :, :], pst,
                                         AF.Relu, bias=0.0, scale=1.0 / 32)

            # ---- phase E: FFN2 + residual + LN2 stats (interleaved) ----
            y2 = sb.tile([P, KO, T], F32, tag="res", bufs=2, name="y2")
            sum2_ps = ps.tile([1, T], F32, tag="acc", bufs=4, name="sum2_ps")
            ssq2_ps = ps.tile([1, T], F32, tag="acc", bufs=4, name="ssq2_ps")
            for mo in range(KO):
                pfull = ps.tile([P, 2, T], F32, tag="mm", bufs=2, name=f"fp{mo}")
                pst = pfull[:, 0, :]
                w2t = sb.tile([P, DFF // 256, 2, P], F8, tag="w2", bufs=2,
                              name=f"w2_{mo}")
                nc.sync.dma_start(w2t, w2q_d[mo])
                for ki in range(DFF // 256):
                    nc.tensor.matmul(pst, lhsT=w2t[:, ki, :, :],
                                     rhs=rT[:, ki, :, :], perf_mode=DR,
                                     start=(ki == 0), stop=False)
                # rank-1: subtract 32*mu1 (broadcast over features) in-psum
                nc.tensor.matmul(pst, lhsT=ones32_1p[0:1, :],
                                 rhs=negmu_row, start=False, stop=True)
                # y2 = rstd1*(ffpsum/32 + xres - mu1) + b2
                nc.vector.scalar_tensor_tensor(out=y2[:, mo, :], in0=pst,
                                               scalar=1.0 / 32, in1=xres[:, mo, :],
                                               op0=OP.mult, op1=OP.add)
                nc.vector.tensor_mul(y2[:, mo, :], y2[:, mo, :], rstd1_sb)
                nc.vector.tensor_scalar_add(y2[:, mo, :], y2[:, mo, :],
                                            b2_t[:, mo:mo + 1])
                # LN2 stats accumulate as chunks complete; the sum chain
                # consumes y2 (f32) directly — no bf16 staging copy
                ysq2 = sb.tile([P, T], BF16, tag="ysq", bufs=2, name=f"ys2_{mo}")
                nc.scalar.activation(ysq2, y2[:, mo, :], AF.Square, bias=0.0,
                                     scale=1.0)
                nc.tensor.matmul(sum2_ps, lhsT=onesf_bcol, rhs=y2[:, mo, :],
                                 start=(mo == 0), stop=(mo == KO - 1))
                nc.tensor.matmul(ssq2_ps, lhsT=ones_bcol, rhs=ysq2,
                                 start=(mo == 0), stop=(mo == KO - 1))

            # ---- phase F: LN2 normalize feature-major + store ----
            # -mu2 and rstd2 rows are PE-broadcast to all 128 partitions via
            # rank-1 matmuls into PSUM; the normalize is then two DVE
            # tensor_tensor ops per chunk and the output DMAs feature-major
            # (the host transposes).
            negmu2 = sb.tile([1, T], F32, tag="lns", bufs=4, name="negmu2")
            nc.scalar.activation(negmu2, sum2_ps, AF.Copy, bias=0.0,
                                 scale=-1.0 / D)
            t2m = sb.tile([1, T], F32, tag="lns", bufs=4, name="t2m")
            nc.vector.tensor_mul(t2m, negmu2, negmu2)
            var2 = sb.tile([1, T], F32, tag="lns", bufs=4, name="var2")
            nc.vector.scalar_tensor_tensor(out=var2, in0=ssq2_ps, scalar=1.0 / D,
                                           in1=t2m, op0=OP.mult, op1=OP.subtract)
            std2 = sb.tile([1, T], F32, tag="lns", bufs=4, name="std2")
            nc.scalar.activation(std2, var2, AF.Sqrt, bias=eps_t[0:1, 0:1],
                                 scale=1.0)
            rstd2 = sb.tile([1, T], F32, tag="lns", bufs=4, name="rstd2")
            nc.vector.reciprocal(rstd2, std2)
            nm2_ps = ps.tile([P, T], F32, tag="acc", bufs=4, name="nm2_ps")
            rs2_ps = ps.tile([P, T], F32, tag="acc", bufs=4, name="rs2_ps")
            nc.tensor.matmul(nm2_ps, lhsT=onesf_1p, rhs=negmu2,
                             start=True, stop=True)
            nc.tensor.matmul(rs2_ps, lhsT=onesf_1p, rhs=rstd2,
                             start=True, stop=True)
            out_r = out_d[:, :].rearrange("(o p) t -> p o t", p=P)
            for mo in range(KO):
                y2n = sb.tile([P, T], F32, tag="scr", bufs=3, name=f"y2n_{mo}")
                nc.vector.tensor_add(y2n, y2[:, mo, :], nm2_ps)
                nc.vector.tensor_mul(y2n, y2n, rs2_ps)
                # scalar (ACT) is idle at the tail; both HWDGE queues share
                # the output so the last chunk lands as early as possible
                (nc.sync, nc.scalar)[mo % 2].dma_start(out_r[:, mo, :], y2n)

    nc.finalize()
    return nc


def _maybe_enable_ldw_opt():
    if os.environ.get("BASS_LDW_OPT") != "1":
        return
    import concourse.bass_utils as _bu
    if getattr(_bu, "_ldw_opt_patched", False):
        return
    _orig = _bu.run_command

    def _patched(argv, **kw):
        argv = ["--enable-ldw-opt=true" if a == "--enable-ldw-opt=false" else a
                for a in argv]
        return _orig(argv, **kw)

    _bu.run_command = _patched
    _bu._ldw_opt_patched = True


_maybe_enable_ldw_opt()

_PROG = None
_last_results = None


def _get_prog():
    global _PROG
    if _PROG is None:
        _PROG = build_program()
    return _PROG


def pack_consts(bq, bk, b1, b2, KO=D_MODEL // P, FO=D_FF // P):
    cols = []
    for vec, n in ((bq, KO), (bk, KO), (b1, FO), (b2, KO)):
        cols.append(np.asarray(vec, np.float32).reshape(n, P).T)  # [P, n]
    return np.ascontiguousarray(np.concatenate(cols, axis=1))


def make_in_maps(x, Wq, bq, Wk, bk, Wv, bv, Wo, bo, W1, b1, W2, b2,
                 ln1_g, ln1_b, ln2_g, ln2_b):
    bf = ml_dtypes.bfloat16
    f32 = np.float32
    x = np.asarray(x, f32)
    f8 = ml_dtypes.float8_e4m3
    W1f = np.asarray(W1, f32)
    c1r = (32.0 * np.stack([W1f.sum(axis=0), np.asarray(b1, f32)])).astype(bf)

    def pack_dr(w):  # [K, M] -> [P, K//256, 2, M] fp8, pre-scaled by 32
        K, M = w.shape
        wi = (np.asarray(w, f32) * 32.0).reshape(K // 256, 2, P, M)
        return np.ascontiguousarray(wi.transpose(2, 0, 1, 3).astype(f8))

    w2q = pack_dr(np.asarray(W2, f32))          # [P, DFF//256, 2, D]
    # repack per-output-chunk contiguous: [KO, P, DFF//256, 2, P]
    w2q = np.ascontiguousarray(
        w2q.reshape(P, D_FF // 256, 2, D_MODEL // P, P).transpose(3, 0, 1, 2, 4))

    shared = {
        "Wq": pack_dr(np.asarray(Wq, f32)),
        "Wk": pack_dr(np.asarray(Wk, f32)),
        "Wv": pack_dr(np.asarray(Wv, f32)),
        "Wo": np.ascontiguousarray(np.asarray(Wo, f32).astype(bf)),
        "w1q": np.ascontiguousarray(
            pack_dr(W1f).reshape(P, D_MODEL // 256, 2, D_FF // P, P)
            .transpose(0, 3, 1, 2, 4)),
        "w2q": w2q,
        "cpk": pack_consts(bq, bk, b1, b2),
        "c1r": np.ascontiguousarray(c1r),
    }
    # bv is invariant under softmax averaging: attn(v + bv) = attn(v) + bv,
    # so fold bv@Wo + bo into the residual once on the host (exact, f32).
    res_bias = (np.asarray(bv, f32) @ np.asarray(Wo, f32)
                + np.asarray(bo, f32))

    def pack_act(a):  # [D, Ntok] -> [P, D//256, 2, Ntok] fp8 interleaved
        Dd, Nt = a.shape
        return np.ascontiguousarray(
            a.reshape(Dd // 256, 2, P, Nt).transpose(2, 0, 1, 3).astype(f8))

    in_maps = []
    xT_by_batch = [np.ascontiguousarray(x[b].T) for b in range(x.shape[0])]
    xTq_by_batch = [pack_act(t) for t in xT_by_batch]
    for c in range(N_CORES):
        b, q0 = c // 4, (c % 4) * TQ
        xslice = xT_by_batch[b][:, q0:q0 + TQ]
        m = dict(shared)
        m["xT"] = xTq_by_batch[b]
        m["xTq"] = np.ascontiguousarray(xTq_by_batch[b][:, :, :, q0:q0 + TQ])
        m["xres"] = np.ascontiguousarray(xslice + res_bias[:, None])
        in_maps.append(m)
    return in_maps


def kernel(**inputs):
    global _last_results
    nc = _get_prog()
    in_maps = make_in_maps(**inputs)
    res = run_bass_kernel_spmd(nc, in_maps, core_ids=list(range(N_CORES)),
                               tmpdir=os.environ.get("BASS_KERNEL_TMPDIR"))
    _last_results = res
    x = np.asarray(inputs["x"])
    B, S, D = x.shape
    out = np.empty((B, S, D), np.float32)
    for c in range(N_CORES):
        b, q0 = c // 4, (c % 4) * TQ
        out[b, q0:q0 + TQ, :] = res.results[c]["out"].T
    return out


# revision 63
# speedup vs baseline: 1.2204x; 1.0217x over previous
"""Trainium2 Bass kernel for a dense transformer encoder layer.

Model (fp32 reference):
    q,k,v = x@Wq+bq, x@Wk+bk, x@Wv+bv          (16 heads, d_k=64)
    attn  = softmax(q k^T / 8) v
    h     = LN(x + attn@Wo + bo)
    out   = LN(h + relu(h@W1+b1)@W2 + b2)      (ln gamma=1, beta=0)

Sharding: query-parallel over 8 cores. Core c handles batch b=c//4,
query rows (c%4)*512..+512. Each core recomputes K/V for its batch's
full 2048-token sequence (no collectives needed); host concatenates the
8 [512, 1024] output slices (the device writes feature-major; the host
transposes).

On-device layout: activations feature-major ([feature, token]) end to
end; scores transposed ([k_tok, q]) so softmax denominators come free
from a ones-column appended to V.

Precision: all five projection groups run as fp8-e4m3 DoubleRow
matmuls (weights host-prescaled by 32, 1/32 folded into evictions).
Scores/ctx stay bf16. bv is exact under softmax averaging, so V is
projected bias-free and bv@Wo+bo is folded into xres on the host.

Engine discipline learned from traces: a dma_start OCCUPIES its
issuing engine for the whole transfer, so the scalar (ACT) engine —
the attention-exp bottleneck — issues no DMAs before the output tail;
sync (HWDGE) and gpsimd (SWDGE) split all traffic as whole-tensor
transfers ordered by first use. W2 is host-repacked per-output-chunk
contiguous and streamed as one 512KB sync DMA per chunk. Attention
emits pair p's normalize+out-projection after pair p+1's score/ctx
matmuls (the PE stream is static, so this keeps the exp pipeline
dense while the all-DVE normalize drains); the attention exp stream
runs within ~6% of the 134us ScalarE floor.

LayerNorm 1 is never materialized: FFN1 consumes fp8(xres) with a K=2
rank-1 fold per chain adding (-mu)(x)colsum(W1) + std(x)b1; relu runs
in the std-scaled domain and rstd1 multiplies at the FFN2 eviction
(which also carries a rank-1 -mu1 fold); both halves of each FFN1
psum tile evict through ONE batched 1024-col relu, halving the
ACT-eviction handshakes. LN1/LN2 std comes from ACT
Sqrt (one table switch, in the post-attention trough) and rstd from
the DVE reciprocal; the fold rows ship as two plain partition-
targeted DMAs (a single partition-expanding transfer raced its
completion semaphore). LN2 statistics accumulate inside the FFN2
loop (the sum chain reads y2 f32 directly); -mu2/rstd2 broadcast via
rank-1 PE matmuls into PSUM and the normalize is two DVE ops per
feature-major chunk with per-chunk output DMAs on both idle HWDGE
queues.
"""

import os

import numpy as np
import ml_dtypes

import concourse.bass as bass
import concourse.bacc as bacc_mod
import concourse.hw_specs as hw_specs
import concourse.tile as tile
import concourse.mybir as mybir
from concourse.bass_utils import run_bass_kernel_spmd

BF16 = mybir.dt.bfloat16
F32 = mybir.dt.float32
F32R = mybir.dt.float32r
F8 = mybir.dt.float8e4
I16 = mybir.dt.int16
DR = mybir.MatmulPerfMode.DoubleRow
AF = mybir.ActivationFunctionType
OP = mybir.AluOpType

P = 128
EPS = 1e-5

# full-problem dims
D_MODEL = 1024
D_FF = 4096
N_HEADS = 16
D_K = 64
SEQ = 2048
TQ = 512          # queries per core
N_CORES = 8


def build_program(D=D_MODEL, DFF=D_FF, H=N_HEADS, S=SEQ, T=TQ):
    """Emit the per-core Bass program (SPMD: same NEFF on all cores)."""
    KO = D // P            # feature chunks of d_model
    FO = DFF // P          # feature chunks of d_ff
    TC = S // P            # key-token chunks
    HP = H // 2            # head pairs (even head on partitions 0-63, odd on 64-127)
    VW = 65                # v-aug row width: 64 v cols + ones col
    WS = min(512, D)       # weight-stream chunk width
    SC = min(512, S)       # score/psum free chunk width
    MI = WS // P
    assert H * D_K == D and TC % 4 == 0 and T <= 512

    nc = bacc_mod.Bacc()

    xT_d = nc.dram_tensor("xT", (P, D // 256, 2, S), F8, kind="ExternalInput")
    xTq_d = nc.dram_tensor("xTq", (P, D // 256, 2, T), F8, kind="ExternalInput")
    xres_d = nc.dram_tensor("xres", (D, T), F32, kind="ExternalInput")
    Wq_d = nc.dram_tensor("Wq", (P, D // 256, 2, D), F8, kind="ExternalInput")
    Wk_d = nc.dram_tensor("Wk", (P, D // 256, 2, D), F8, kind="ExternalInput")
    Wv_d = nc.dram_tensor("Wv", (P, D // 256, 2, D), F8, kind="ExternalInput")
    Wo_d = nc.dram_tensor("Wo", (D, D), BF16, kind="ExternalInput")
    # fp8 DoubleRow-interleaved FFN weights, pre-scaled by 32 on the host
    # W1 packed per-output-chunk contiguous ([P][fo][K//256][2][P]) so the
    # DR LDWEIGHTS source slices are small-stride like W2's
    w1q_d = nc.dram_tensor("w1q", (P, DFF // P, D // 256, 2, P), F8,
                           kind="ExternalInput")
    # W2 packed per-output-chunk contiguous: [KO][P, DFF//256, 2, P]
    w2q_d = nc.dram_tensor("w2q", (KO, P, DFF // 256, 2, P), F8,
                           kind="ExternalInput")
    # packed per-partition biases: [bq | bk | b1 | b2] as [P, KO+KO+FO+KO]
    cpk_d = nc.dram_tensor("cpk", (P, 3 * KO + FO), F32, kind="ExternalInput")
    # rank-1 LN1-fold rows: [colsum(W1) ; b1] as [2, DFF] bf16
    c1r_d = nc.dram_tensor("c1r", (2, DFF), BF16, kind="ExternalInput")
    # output is written feature-major [D, T]; the host transposes
    out_d = nc.dram_tensor("out", (D, T), F32, kind="ExternalOutput")

    def wr(w):  # [K, M] weight dram -> [P, K//P, M] partition-chunked view
        return w[:, :].rearrange("(o p) m -> p o m", p=P)

    with tile.TileContext(nc) as tc:
        with (
            tc.tile_pool(name="sb", bufs=1) as sb,
            tc.tile_pool(name="ps", bufs=1, space="PSUM") as ps,
        ):
            # ---- phase A: projections; q first so the PE starts early ----
            KO2 = KO // 2
            xTq = sb.tile([P, KO2, 2, T], F8, tag="mid", bufs=2, name="xTq")
            qT = sb.tile([P, KO, T], BF16, tag="mid", bufs=2, name="qT")
            # PE warm-up: HAM releases the clock gate after ~3.4us of activity.
            # Junk matmuls on not-yet-written qT (never read back) span the
            # first input DMAs so the real chains start at full clock.
            wps = ps.tile([P, 2, T], F32, tag="mm", bufs=2, name="wps")
            for i in range(5):
                nc.tensor.matmul(wps[:, 0, :], lhsT=qT[:, 0, 0:P], rhs=qT[:, 0, :],
                                 start=(i == 0), stop=(i == 4))
            xT = sb.tile([P, KO2, 2, S], F8, tag="big", bufs=3, name="xT")
            kT = sb.tile([P, KO, S], BF16, tag="big", bufs=3, name="kT")
            vAug = sb.tile([P, TC, H, VW], BF16, tag="big", bufs=3, name="vAug")
            cpk = sb.tile([P, 3 * KO + FO], F32, name="cpk")
            bq_t, bk_t = cpk[:, 0:KO], cpk[:, KO:2 * KO]
            b2_t = cpk[:, 2 * KO + FO:]
            c1r_t = sb.tile([2, DFF], BF16, name="c1r_t")
            onesf_1p = sb.tile([1, P], F32, name="onesf_1p")
            onesf_bcol = sb.tile([P, 1], F32, name="onesf_bcol")
            ones32_1p = sb.tile([1, P], BF16, name="ones32_1p")
            ones_bcol = sb.tile([P, 1], BF16, name="ones_bcol")
            eps_t = sb.tile([1, 1], F32, name="eps_t")

            # startup DMAs: scalar (ACT) issues NO dmas — a dma_start occupies
            # its engine for the whole transfer and ACT is the exp bottleneck.
            # gpsimd (SWDGE) and sync (HWDGE) split the traffic, ordered by
            # first use.
            w4 = S // 4
            wq0 = sb.tile([P, KO2, 2, WS], F8, tag="wst", bufs=3, name="wq0")
            wq1 = sb.tile([P, KO2, 2, WS], F8, tag="wst", bufs=3, name="wq1")
            nc.gpsimd.dma_start(xTq, xTq_d[:, :, :, :])
            nc.sync.dma_start(wq0, Wq_d[:, :, :, 0:WS])
            nc.gpsimd.dma_start(cpk, cpk_d[:, :])
            nc.sync.dma_start(wq1, Wq_d[:, :, :, WS:2 * WS])
            nc.gpsimd.dma_start(xT[:, :, :, 0:w4], xT_d[:, :, :, 0:w4])
            nc.gpsimd.dma_start(xT[:, :, :, 2 * w4:3 * w4],
                                xT_d[:, :, :, 2 * w4:3 * w4])
            nc.vector.memset(onesf_1p, 1.0)
            nc.vector.memset(onesf_bcol, 1.0)
            nc.vector.memset(ones32_1p, 32.0)
            nc.vector.memset(ones_bcol, 1.0)
            nc.vector.memset(eps_t, EPS)
            # touch ACT immediately so its one-time table load (~2.7us with
            # drain) runs during the initial DMA wait instead of stalling the
            # first Q eviction
            nc.scalar.activation(eps_t, eps_t, AF.Copy, bias=0.0, scale=1.0)

            for mo2 in range(D // WS):
                wt = (wq0, wq1)[mo2]
                for mi in range(0, MI, 2):
                    pst = ps.tile([P, 2, T], F32, tag="mm", bufs=2, name=f"qp{mo2}_{mi}")
                    for half in range(2):
                        mo = mo2 * MI + mi + half
                        msl = slice((mi + half) * P, (mi + half + 1) * P)
                        for o2 in range(KO2):
                            nc.tensor.matmul(
                                pst[:, half, :],
                                lhsT=wt[:, o2, :, msl],
                                rhs=xTq[:, o2, :, :], perf_mode=DR,
                                start=(o2 == 0), stop=(o2 == KO2 - 1))
                        nc.scalar.activation(qT[:, mo, :], pst[:, half, :], AF.Identity,
                                             bias=bq_t[:, mo:mo + 1], scale=1.0 / 32)

            # k^T [D, S]: two token-chunk chains per psum tile, one batched evict
            for mo2 in range(D // WS):
                wt = sb.tile([P, KO2, 2, WS], F8, tag="wst", bufs=3, name=f"wk{mo2}")
                nc.sync.dma_start(wt, Wk_d[:, :, :, mo2 * WS:(mo2 + 1) * WS])
                if mo2 == 0:
                    nc.sync.dma_start(xT[:, :, :, w4:2 * w4],
                                      xT_d[:, :, :, w4:2 * w4])
                    nc.sync.dma_start(xT[:, :, :, 3 * w4:4 * w4],
                                      xT_d[:, :, :, 3 * w4:4 * w4])
                for mi in range(MI):
                    mo = mo2 * MI + mi
                    n_ch = S // SC
                    for nc2 in range((n_ch + 1) // 2):
                        w = min(2, n_ch - nc2 * 2)
                        pst = ps.tile([P, 2, SC], F32, tag="mm", bufs=2,
                                      name=f"kp{mo}_{nc2}")
                        for half in range(w):
                            ncc = nc2 * 2 + half
                            for o2 in range(KO2):
                                nc.tensor.matmul(pst[:, half, :],
                                                 lhsT=wt[:, o2, :, mi * P:(mi + 1) * P],
                                                 rhs=xT[:, o2, :, ncc * SC:(ncc + 1) * SC],
                                                 perf_mode=DR,
                                                 start=(o2 == 0), stop=(o2 == KO2 - 1))
                        nc.scalar.activation(
                            kT[:, mo, nc2 * 2 * SC:nc2 * 2 * SC + w * SC],
                            pst[:, 0:w, :], AF.Identity,
                            bias=bk_t[:, mo:mo + 1], scale=1.0 / 32)

            xres = sb.tile([P, KO, T], F32, tag="res", bufs=2, name="xres")

            # v token-major [S, D] with appended ones column per head:
            # vAug[p, tc, h, 0:64] = v[tc*128+p, h*64:(h+1)*64],  vAug[.., 64] = 1
            # (bv is folded into xres on the host: softmax weights sum to 1)
            # Evictions run on DVE so ACT stays exp-only during attention.
            nc.vector.memset(vAug[:, :, :, D_K:D_K + 1], 1.0)
            wv0 = sb.tile([P, KO2, 2, WS], F8, tag="wst", bufs=3, name="wv0")
            wv1 = sb.tile([P, KO2, 2, WS], F8, tag="wst", bufs=3, name="wv1")
            nc.sync.dma_start(wv0, Wv_d[:, :, :, 0:WS])
            nc.sync.dma_start(wv1, Wv_d[:, :, :, WS:2 * WS])
            # xres (2MB, first needed ~10us into attention) rides the SYNC
            # ring BEHIND all projection weights: engine DMA queues run ahead
            # of program order, so on gpsimd this transfer fired mid-K-phase
            # and starved the weight streams of aggregate SDMA bandwidth
            # (wk0 was observed arriving 17us after issue)
            nc.sync.dma_start(xres, xres_d[:, :].rearrange("(o p) t -> p o t", p=P))
            nh = WS // D_K

            def v_chunk(no2, tc_, wt):
                pfull = ps.tile([P, 2, SC], F32, tag="mm", bufs=2,
                                name=f"vp{no2}_{tc_}")
                pst = pfull[:, 0, :WS]
                for o2 in range(KO2):
                    nc.tensor.matmul(pst,
                                     lhsT=xT[:, o2, :, tc_ * P:(tc_ + 1) * P],
                                     rhs=wt[:, o2, :, :], perf_mode=DR,
                                     start=(o2 == 0), stop=(o2 == KO2 - 1))
                nc.scalar.activation(
                    vAug[:, tc_, no2 * nh:(no2 + 1) * nh, 0:D_K],
                    pst.rearrange("p (h d) -> p h d", d=D_K),
                    AF.Copy, bias=0.0, scale=1.0 / 32)

            for tc_ in range(TC):
                v_chunk(0, tc_, wv0)
            for tc_ in range(TC):
                v_chunk(1, tc_, wv1)

            # W1's 4MB preload is gated on xT's big-ring slot being released
            # by the last V chain, so it cannot contend with the startup
            nc.gpsimd.dma_start(c1r_t, c1r_d[:, :])
            w1q = sb.tile([P, DFF // P, D // 256, 2, P], F8, tag="big", bufs=3,
                          name="w1q")
            nc.gpsimd.dma_start(w1q, w1q_d[:, :, :, :])

            def bcast_prep(t):
                # stream_shuffle streams all 32 input lanes; zero the quadrant
                # BEFORE the row-0 write so nothing is read uninitialized.
                nc.vector.memset(t[0:32, :], 0.0)

            def bcast_from_row0(t, rows=128):
                """Replicate t[0:1, :] (SBUF) to partitions 0..rows, DVE-only."""
                nc.vector.stream_shuffle(t[32:64, :], t[0:32, :], mask=[0] * 32)
                nc.vector.tensor_copy(t[0:32, :], t[32:64, :])
                if rows > 64:
                    nc.vector.tensor_copy(t[64:96, :], t[32:64, :])
                    nc.vector.tensor_copy(t[96:128, :], t[32:64, :])

            # ---- phase B: attention with fused out-projection ----
            # Per kc chunk both heads of a pair land in ONE [P, 2, T] psum tile
            # so a single ACT exp covers 1024 elements. ctx row 64 = softmax
            # denominator (ones column of vAug). ctx accumulators live in a
            # 3-deep "cps" ring (pair p+1 never waits on pair p's normalize);
            # pair p's out-projection matmuls interleave into pair p+1's kc
            # loop against a dedicated 1-bank "op" ring, each eviction a DVE
            # add into xres.
            def norm_muls(hp, cpsA, cpsB):
                ctxp = sb.tile([P, T], BF16, tag="ctxp", bufs=2, name=f"cx{hp}")
                for h, cps in ((2 * hp, cpsA), (2 * hp + 1, cpsB)):
                    base = D_K * (h % 2)
                    bcs = sb.tile([P, T], F32, tag="scr", bufs=3, name=f"bc{h}")
                    bcast_prep(bcs)
                    # custom-DVE ops mis-read PSUM: stage the denominator row
                    # into SBUF first, then reciprocal in place.
                    nc.vector.tensor_copy(bcs[0:1, :], cps[D_K:D_K + 1, :])
                    nc.vector.reciprocal_approx_fast(bcs[0:1, :], bcs[0:1, :])
                    bcast_from_row0(bcs, rows=64)
                    nc.vector.tensor_mul(ctxp[base:base + D_K, :],
                                         cps[0:D_K, :], bcs[0:D_K, :])
                return ctxp

            def emit_op(hp, ctxp, wo_t, mo):
                # acc-ring op tile: during pair p's loop the ring holds the
                # two open ctx accumulators plus two free slots (pair p-1's,
                # released by its norm_muls at the top of pair p); the
                # allocator hands out free slots, so these never touch the
                # open accumulators or the score-tile (mm) ring.
                op = ps.tile([P, T], F32, tag="acc", bufs=4, name=f"o{hp}_{mo}")
                nc.tensor.matmul(op, lhsT=wo_t[:, mo * P:(mo + 1) * P],
                                 rhs=ctxp, start=True, stop=True)
                nc.vector.tensor_add(xres[:, mo, :], op, xres[:, mo, :])

            def op_pair(wo_t, ctxp, mg, name):
                """Two out-proj chunks into one mm-ring tile + one DVE add."""
                op2 = ps.tile([P, 2, T], F32, tag="mm", bufs=2, name=name)
                for half in range(2):
                    mo = 2 * mg + half
                    nc.tensor.matmul(op2[:, half, :],
                                     lhsT=wo_t[:, mo * P:(mo + 1) * P],
                                     rhs=ctxp, start=True, stop=True)
                xr2 = xres[:, 2 * mg:2 * mg + 2, :]
                nc.vector.tensor_add(xr2, op2, xr2)
                return xr2

            # Pair p-1's normalize (all-DVE) is emitted at pair p's top and
            # runs under the first ~6 exps; its 8 out-proj matmuls then
            # interleave one-per-2-kc from kc=6 (5 in-loop, 3 at the
            # boundary), filling PE slack in the ACT-bound steady state so
            # only pair 7's normalize+out-proj remains after the last exp.
            pend = None
            opnd = None
            for hp in range(HP):
                hA, hB = 2 * hp, 2 * hp + 1
                wo_t = sb.tile([P, D], BF16, tag="wo", bufs=2, name=f"wo{hp}")
                nc.sync.dma_start(wo_t, wr(Wo_d)[:, hp, :])
                cpsA = ps.tile([P, T], F32, tag="acc", bufs=4, name=f"cA{hp}")
                cpsB = ps.tile([P, T], F32, tag="acc", bufs=4, name=f"cB{hp}")
                if pend is not None:
                    opnd = (pend[0], norm_muls(*pend[:3]), pend[3])
                for kc in range(TC):
                    s2 = ps.tile([P, 2, T], F32, tag="mm", bufs=2, name=f"s{hp}_{kc}")
                    nc.tensor.matmul(s2[:, 0, :],
                                     lhsT=kT[0:D_K, hp, kc * P:(kc + 1) * P],
                                     rhs=qT[0:D_K, hp, :], start=True, stop=True)
                    nc.tensor.matmul(s2[:, 1, :],
                                     lhsT=kT[D_K:P, hp, kc * P:(kc + 1) * P],
                                     rhs=qT[D_K:P, hp, :], start=True, stop=True)
                    e2 = sb.tile([P, 2, T], BF16, tag="e", bufs=4,
                                 name=f"e{hp}_{kc}")
                    nc.scalar.activation(e2, s2, AF.Exp, scale=0.125)
                    nc.tensor.matmul(cpsA[0:D_K + 1, :],
                                     lhsT=vAug[:, kc, hA, 0:D_K + 1],
                                     rhs=e2[:, 0, :],
                                     start=(kc == 0), stop=(kc == TC - 1))
                    nc.tensor.matmul(cpsB[0:D_K + 1, :],
                                     lhsT=vAug[:, kc, hB, 0:D_K + 1],
                                     rhs=e2[:, 1, :],
                                     start=(kc == 0), stop=(kc == TC - 1))
                    if opnd is not None and ((kc >= 6 and kc % 2 == 0)
                                             or kc == 15):
                        j = 5 if kc == 15 else (kc - 6) // 2
                        emit_op(opnd[0], opnd[1], opnd[2], j)
                if opnd is not None:
                    for mo in (6, 7):
                        emit_op(opnd[0], opnd[1], opnd[2], mo)
                    opnd = None
                pend = (hp, cpsA, cpsB, wo_t)

            # ---- pair 7 out-projection with LN1 stats interleaved ----
            # No exp pressure remains: per 2 chunks, two op matmuls into one
            # mm-ring tile, ONE DVE add, ONE 1024-col fp8 quantize, then the
            # LN1 stat chains (LN1 itself is folded into FFN1).
            ctxp7, wo7 = norm_muls(7, cpsA, cpsB), wo_t
            xres_q = sb.tile([P, KO // 2, 2, T], F8, tag="mid", bufs=2, name="xres_q")
            sum1_ps = ps.tile([1, T], F32, tag="acc", bufs=4, name="sum1_ps")
            ssq1_ps = ps.tile([1, T], F32, tag="acc", bufs=4, name="ssq1_ps")
            for mg in range(KO // 2):
                xr2 = op_pair(wo7, ctxp7, mg, f"o7_{mg}")
                xq2 = xres_q[:, mg, :, :]
                nc.scalar.activation(xq2, xr2, AF.Copy, bias=0.0, scale=1.0)
                ysq = sb.tile([P, 2, T], BF16, tag="ysq", bufs=2, name=f"ys1_{mg}")
                nc.vector.tensor_mul(ysq, xq2, xq2)
                for half in range(2):
                    mo = 2 * mg + half
                    nc.tensor.matmul(sum1_ps, lhsT=ones_bcol, rhs=xq2[:, half, :],
                                     start=(mo == 0), stop=(mo == KO - 1))
                    nc.tensor.matmul(ssq1_ps, lhsT=ones_bcol, rhs=ysq[:, half, :],
                                     start=(mo == 0), stop=(mo == KO - 1))

            # ---- phase C: LN1 rows (stats only; LN1 is folded into FFN) ----
            # std via ACT Sqrt (one table switch, stays loaded through LN2),
            # rstd via the standard DVE reciprocal.
            mu1f = sb.tile([1, T], F32, tag="lns", bufs=4, name="mu1f")
            nc.scalar.activation(mu1f, sum1_ps, AF.Copy, bias=0.0, scale=1.0 / D)
            var1 = sb.tile([1, T], F32, tag="lns", bufs=4, name="var1")
            nc.vector.tensor_mul(var1, mu1f, mu1f)
            nc.vector.scalar_tensor_tensor(out=var1, in0=ssq1_ps, scalar=1.0 / D,
                                           in1=var1, op0=OP.mult, op1=OP.subtract)
            std1 = sb.tile([1, T], F32, tag="lns", bufs=4, name="std1")
            nc.scalar.activation(std1, var1, AF.Sqrt, bias=eps_t[0:1, 0:1],
                                 scale=1.0)
            rstd1_sb = sb.tile([P, T], F32, tag="scr", bufs=3, name="rstd1_sb")
            bcast_prep(rstd1_sb)
            nc.vector.reciprocal(rstd1_sb[0:1, :], std1)
            bcast_from_row0(rstd1_sb)
            # pack [-mu ; std] contiguously in one partition-0 stage tile and
            # ship both rows with a single partition-crossing DMA (engine APs
            # can't write partition 1, but DMA descriptors can)
            rstage = sb.tile([1, 2, T], BF16, tag="lns", bufs=4, name="rstage")
            nc.scalar.activation(rstage[0:1, 0, :], sum1_ps, AF.Copy, bias=0.0,
                                 scale=-1.0 / D)
            nc.scalar.activation(rstage[0:1, 1, :], std1, AF.Copy, bias=0.0,
                                 scale=1.0)
            negmu_row = rstage[0:1, 0, :]
            rmix = sb.tile([2, T], BF16, tag="lns", bufs=4, name="rmix")
            # two plain partition-targeted DMAs: the single partition-
            # expanding transfer was observed to race its completion
            # semaphore (stride-4 column groups landing late)
            nc.sync.dma_start(rmix[0:1, :], rstage[0:1, 0, :])
            nc.sync.dma_start(rmix[1:2, :], rstage[0:1, 1, :])

            # ---- phase D: FFN1 + relu (std-scaled domain, no bias on ACT) ----
            # fp8 DoubleRow: 4 K=256 matmuls per chain; psum carries 32x the
            # true value (host-scaled weights), rescaled at the relu eviction.
            rT = sb.tile([P, FO // 2, 2, T], F8, tag="big", bufs=3, name="rT")
            for fo2 in range(DFF // WS):
                for fi in range(0, MI, 2):
                    pst = ps.tile([P, 2, T], F32, tag="mm", bufs=2, name=f"zp{fo2}_{fi}")
                    # run both halves' DR passes as ONE 8-pass fp8 streak and
                    # append the two bf16 rank-1 folds after: each DR<->normal
                    # perf-mode switch costs LDW-pipeline refill, so two
                    # switches per tile instead of four
                    for half in range(2):
                        fo = fo2 * MI + fi + half
                        for o2 in range(KO // 2):
                            nc.tensor.matmul(pst[:, half, :],
                                             lhsT=w1q[:, fo, o2, :, :],
                                             rhs=xres_q[:, o2, :, :],
                                             perf_mode=DR,
                                             start=(o2 == 0), stop=False)
                    for half in range(2):
                        fo = fo2 * MI + fi + half
                        # K=2 rank-1 fold: (-mu)(x)colsum(W1) + std(x)b1
                        nc.tensor.matmul(pst[:, half, :],
                                         lhsT=c1r_t[0:2, fo * P:(fo + 1) * P],
                                         rhs=rmix[0:2, :], start=False, stop=True)
                    # both halves land in the same rT [., fo//2, 0:2, .]
                    # slice: ONE batched 1024-col relu eviction per psum tile
                    fo = fo2 * MI + fi
                    nc.scalar.activation(rT[:, fo // 2, :, :], pst,
                                         AF.Relu, bias=0.0, scale=1.0 / 32)

            # ---- phase E: FFN2 + residual + LN2 stats (interleaved) ----
            y2 = sb.tile([P, KO, T], F32, tag="res", bufs=2, name="y2")
            sum2_ps = ps.tile([1, T], F32, tag="acc", bufs=4, name="sum2_ps")
            ssq2_ps = ps.tile([1, T], F32, tag="acc", bufs=4, name="ssq2_ps")
            for mo in range(KO):
                pfull = ps.tile([P, 2, T], F32, tag="mm", bufs=2, name=f"fp{mo}")
                pst = pfull[:, 0, :]
                w2t = sb.tile([P, DFF // 256, 2, P], F8, tag="w2", bufs=2,
                              name=f"w2_{mo}")
                nc.sync.dma_start(w2t, w2q_d[mo])
                for ki in range(DFF // 256):
                    nc.tensor.matmul(pst, lhsT=w2t[:, ki, :, :],
                                     rhs=rT[:, ki, :, :], perf_mode=DR,
                                     start=(ki == 0), stop=False)
                # rank-1: subtract 32*mu1 (broadcast over features) in-psum
                nc.tensor.matmul(pst, lhsT=ones32_1p[0:1, :],
                                 rhs=negmu_row, start=False, stop=True)
                # y2 = rstd1*(ffpsum/32 + xres - mu1) + b2
                nc.vector.scalar_tensor_tensor(out=y2[:, mo, :], in0=pst,
                                               scalar=1.0 / 32, in1=xres[:, mo, :],
                                               op0=OP.mult, op1=OP.add)
                nc.vector.tensor_mul(y2[:, mo, :], y2[:, mo, :], rstd1_sb)
                nc.vector.tensor_scalar_add(y2[:, mo, :], y2[:, mo, :],
                                            b2_t[:, mo:mo + 1])
                # LN2 stats accumulate as chunks complete; the sum chain
                # consumes y2 (f32) directly — no bf16 staging copy
                ysq2 = sb.tile([P, T], BF16, tag="ysq", bufs=2, name=f"ys2_{mo}")
                nc.scalar.activation(ysq2, y2[:, mo, :], AF.Square, bias=0.0,
                                     scale=1.0)
                nc.tensor.matmul(sum2_ps, lhsT=onesf_bcol, rhs=y2[:, mo, :],
                                 start=(mo == 0), stop=(mo == KO - 1))
                nc.tensor.matmul(ssq2_ps, lhsT=ones_bcol, rhs=ysq2,
                                 start=(mo == 0), stop=(mo == KO - 1))

            # ---- phase F: LN2 normalize feature-major + store ----
            # -mu2 and rstd2 rows are PE-broadcast to all 128 partitions via
            # rank-1 matmuls into PSUM; the normalize is then two DVE
            # tensor_tensor ops per chunk and the output DMAs feature-major
            # (the host transposes).
            negmu2 = sb.tile([1, T], F32, tag="lns", bufs=4, name="negmu2")
            nc.scalar.activation(negmu2, sum2_ps, AF.Copy, bias=0.0,
                                 scale=-1.0 / D)
            t2m = sb.tile([1, T], F32, tag="lns", bufs=4, name="t2m")
            nc.vector.tensor_mul(t2m, negmu2, negmu2)
            var2 = sb.tile([1, T], F32, tag="lns", bufs=4, name="var2")
            nc.vector.scalar_tensor_tensor(out=var2, in0=ssq2_ps, scalar=1.0 / D,
                                           in1=t2m, op0=OP.mult, op1=OP.subtract)
            std2 = sb.tile([1, T], F32, tag="lns", bufs=4, name="std2")
            nc.scalar.activation(std2, var2, AF.Sqrt, bias=eps_t[0:1, 0:1],
                                 scale=1.0)
            rstd2 = sb.tile([1, T], F32, tag="lns", bufs=4, name="rstd2")
            nc.vector.reciprocal(rstd2, std2)
            nm2_ps = ps.tile([P, T], F32, tag="acc", bufs=4, name="nm2_ps")
            rs2_ps = ps.tile([P, T], F32, tag="acc", bufs=4, name="rs2_ps")
            nc.tensor.matmul(nm2_ps, lhsT=onesf_1p, rhs=negmu2,
                             start=True, stop=True)
            nc.tensor.matmul(rs2_ps, lhsT=onesf_1p, rhs=rstd2,
                             start=True, stop=True)
            out_r = out_d[:, :].rearrange("(o p) t -> p o t", p=P)
            for mo in range(KO):
                y2n = sb.tile([P, T], F32, tag="scr", bufs=3, name=f"y2n_{mo}")
                nc.vector.tensor_add(y2n, y2[:, mo, :], nm2_ps)
                nc.vector.tensor_mul(y2n, y2n, rs2_ps)
                # scalar (ACT) is idle at the tail; both HWDGE queues share
                # the output so the last chunk lands as early as possible
                (nc.sync, nc.scalar)[mo % 2].dma_start(out_r[:, mo, :], y2n)

    nc.finalize()
    return nc


def _maybe_enable_ldw_opt():
    if os.environ.get("BASS_LDW_OPT") != "1":
        return
    import concourse.bass_utils as _bu
    if getattr(_bu, "_ldw_opt_patched", False):
        return
    _orig = _bu.run_command

    def _patched(argv, **kw):
        argv = ["--enable-ldw-opt=true" if a == "--enable-ldw-opt=false" else a
                for a in argv]
        return _orig(argv, **kw)

    _bu.run_command = _patched
    _bu._ldw_opt_patched = True


_maybe_enable_ldw_opt()

_PROG = None
_last_results = None


def _get_prog():
    global _PROG
    if _PROG is None:
        _PROG = build_program()
    return _PROG


def pack_consts(bq, bk, b1, b2, KO=D_MODEL // P, FO=D_FF // P):
    cols = []
    for vec, n in ((bq, KO), (bk, KO), (b1, FO), (b2, KO)):
        cols.append(np.asarray(vec, np.float32).reshape(n, P).T)  # [P, n]
    return np.ascontiguousarray(np.concatenate(cols, axis=1))


def make_in_maps(x, Wq, bq, Wk, bk, Wv, bv, Wo, bo, W1, b1, W2, b2,
                 ln1_g, ln1_b, ln2_g, ln2_b):
    bf = ml_dtypes.bfloat16
    f32 = np.float32
    x = np.asarray(x, f32)
    f8 = ml_dtypes.float8_e4m3
    W1f = np.asarray(W1, f32)
    c1r = (32.0 * np.stack([W1f.sum(axis=0), np.asarray(b1, f32)])).astype(bf)

    def pack_dr(w):  # [K, M] -> [P, K//256, 2, M] fp8, pre-scaled by 32
        K, M = w.shape
        wi = (np.asarray(w, f32) * 32.0).reshape(K // 256, 2, P, M)
        return np.ascontiguousarray(wi.transpose(2, 0, 1, 3).astype(f8))

    w2q = pack_dr(np.asarray(W2, f32))          # [P, DFF//256, 2, D]
    # repack per-output-chunk contiguous: [KO, P, DFF//256, 2, P]
    w2q = np.ascontiguousarray(
        w2q.reshape(P, D_FF // 256, 2, D_MODEL // P, P).transpose(3, 0, 1, 2, 4))

    shared = {
        "Wq": pack_dr(np.asarray(Wq, f32)),
        "Wk": pack_dr(np.asarray(Wk, f32)),
        "Wv": pack_dr(np.asarray(Wv, f32)),
        "Wo": np.ascontiguousarray(np.asarray(Wo, f32).astype(bf)),
        "w1q": np.ascontiguousarray(
            pack_dr(W1f).reshape(P, D_MODEL // 256, 2, D_FF // P, P)
            .transpose(0, 3, 1, 2, 4)),
        "w2q": w2q,
        "cpk": pack_consts(bq, bk, b1, b2),
        "c1r": np.ascontiguousarray(c1r),
    }
    # bv is invariant under softmax averaging: attn(v + bv) = attn(v) + bv,
    # so fold bv@Wo + bo into the residual once on the host (exact, f32).
    res_bias = (np.asarray(bv, f32) @ np.asarray(Wo, f32)
                + np.asarray(bo, f32))

    def pack_act(a):  # [D, Ntok] -> [P, D//256, 2, Ntok] fp8 interleaved
        Dd, Nt = a.shape
        return np.ascontiguousarray(
            a.reshape(Dd // 256, 2, P, Nt).transpose(2, 0, 1, 3).astype(f8))

    in_maps = []
    xT_by_batch = [np.ascontiguousarray(x[b].T) for b in range(x.shape[0])]
    xTq_by_batch = [pack_act(t) for t in xT_by_batch]
    for c in range(N_CORES):
        b, q0 = c // 4, (c % 4) * TQ
        xslice = xT_by_batch[b][:, q0:q0 + TQ]
        m = dict(shared)
        m["xT"] = xTq_by_batch[b]
        m["xTq"] = np.ascontiguousarray(xTq_by_batch[b][:, :, :, q0:q0 + TQ])
        m["xres"] = np.ascontiguousarray(xslice + res_bias[:, None])
        in_maps.append(m)
    return in_maps


def kernel(**inputs):
    global _last_results
    nc = _get_prog()
    in_maps = make_in_maps(**inputs)
    res = run_bass_kernel_spmd(nc, in_maps, core_ids=list(range(N_CORES)),
                               tmpdir=os.environ.get("BASS_KERNEL_TMPDIR"))
    _last_results = res
    x = np.asarray(inputs["x"])
    B, S, D = x.shape
    out = np.empty((B, S, D), np.float32)
    for c in range(N_CORES):
        b, q0 = c // 4, (c % 4) * TQ
        out[b, q0:q0 + TQ, :] = res.results[c]["out"].T
    return out


# revision 64
# speedup vs baseline: 1.2206x; 1.0002x over previous
"""Trainium2 Bass kernel for a dense transformer encoder layer.

Model (fp32 reference):
    q,k,v = x@Wq+bq, x@Wk+bk, x@Wv+bv          (16 heads, d_k=64)
    attn  = softmax(q k^T / 8) v
    h     = LN(x + attn@Wo + bo)
    out   = LN(h + relu(h@W1+b1)@W2 + b2)      (ln gamma=1, beta=0)

Sharding: query-parallel over 8 cores. Core c handles batch b=c//4,
query rows (c%4)*512..+512. Each core recomputes K/V for its batch's
full 2048-token sequence (no collectives needed); host concatenates the
8 [512, 1024] output slices (the device writes feature-major; the host
transposes).

On-device layout: activations feature-major ([feature, token]) end to
end; scores transposed ([k_tok, q]) so softmax denominators come free
from a ones-column appended to V.

Precision: all five projection groups run as fp8-e4m3 DoubleRow
matmuls (weights host-prescaled by 32, 1/32 folded into evictions).
Scores/ctx stay bf16. bv is exact under softmax averaging, so V is
projected bias-free and bv@Wo+bo is folded into xres on the host.

Engine discipline learned from traces: a dma_start OCCUPIES its
issuing engine for the whole transfer, so the scalar (ACT) engine —
the attention-exp bottleneck — issues no DMAs before the output tail;
sync (HWDGE) and gpsimd (SWDGE) split all traffic as whole-tensor
transfers ordered by first use. W2 is host-repacked per-output-chunk
contiguous and streamed as one 512KB sync DMA per chunk. Attention
emits pair p's normalize+out-projection after pair p+1's score/ctx
matmuls (the PE stream is static, so this keeps the exp pipeline
dense while the all-DVE normalize drains); the attention exp stream
runs within ~6% of the 134us ScalarE floor.

LayerNorm 1 is never materialized: FFN1 consumes fp8(xres) with a K=2
rank-1 fold per chain adding (-mu)(x)colsum(W1) + std(x)b1; relu runs
in the std-scaled domain and rstd1 multiplies at the FFN2 eviction
(which also carries a rank-1 -mu1 fold); each FFN1 psum tile runs
both halves' fp8-DR passes as one 8-pass streak before its two bf16
folds (each DR<->normal perf-mode switch costs LDW-pipeline refill —
fold-per-half paced 267ns/pass vs ~216 for pure DR streaks) and
evicts through ONE batched 1024-col relu, halving the ACT-eviction
handshakes. LN1/LN2 std comes from ACT
Sqrt (one table switch, in the post-attention trough) and rstd from
the DVE reciprocal; the fold rows ship as two plain partition-
targeted DMAs (a single partition-expanding transfer raced its
completion semaphore). LN2 statistics accumulate inside the FFN2
loop (the sum chain reads y2 f32 directly); -mu2/rstd2 broadcast via
rank-1 PE matmuls into PSUM and the normalize is two DVE ops per
feature-major chunk with per-chunk output DMAs on both idle HWDGE
queues.
"""

import os

import numpy as np
import ml_dtypes

import concourse.bass as bass
import concourse.bacc as bacc_mod
import concourse.hw_specs as hw_specs
import concourse.tile as tile
import concourse.mybir as mybir
from concourse.bass_utils import run_bass_kernel_spmd

BF16 = mybir.dt.bfloat16
F32 = mybir.dt.float32
F32R = mybir.dt.float32r
F8 = mybir.dt.float8e4
I16 = mybir.dt.int16
DR = mybir.MatmulPerfMode.DoubleRow
AF = mybir.ActivationFunctionType
OP = mybir.AluOpType

P = 128
EPS = 1e-5

# full-problem dims
D_MODEL = 1024
D_FF = 4096
N_HEADS = 16
D_K = 64
SEQ = 2048
TQ = 512          # queries per core
N_CORES = 8


def build_program(D=D_MODEL, DFF=D_FF, H=N_HEADS, S=SEQ, T=TQ):
    """Emit the per-core Bass program (SPMD: same NEFF on all cores)."""
    KO = D // P            # feature chunks of d_model
    FO = DFF // P          # feature chunks of d_ff
    TC = S // P            # key-token chunks
    HP = H // 2            # head pairs (even head on partitions 0-63, odd on 64-127)
    VW = 65                # v-aug row width: 64 v cols + ones col
    WS = min(512, D)       # weight-stream chunk width
    SC = min(512, S)       # score/psum free chunk width
    MI = WS // P
    assert H * D_K == D and TC % 4 == 0 and T <= 512

    nc = bacc_mod.Bacc()

    xT_d = nc.dram_tensor("xT", (P, D // 256, 2, S), F8, kind="ExternalInput")
    xTq_d = nc.dram_tensor("xTq", (P, D // 256, 2, T), F8, kind="ExternalInput")
    xres_d = nc.dram_tensor("xres", (D, T), F32, kind="ExternalInput")
    Wq_d = nc.dram_tensor("Wq", (P, D // 256, 2, D), F8, kind="ExternalInput")
    Wk_d = nc.dram_tensor("Wk", (P, D // 256, 2, D), F8, kind="ExternalInput")
    Wv_d = nc.dram_tensor("Wv", (P, D // 256, 2, D), F8, kind="ExternalInput")
    Wo_d = nc.dram_tensor("Wo", (D, D), BF16, kind="ExternalInput")
    # fp8 DoubleRow-interleaved FFN weights, pre-scaled by 32 on the host
    # W1 packed per-output-chunk contiguous ([P][fo][K//256][2][P]) so the
    # DR LDWEIGHTS source slices are small-stride like W2's
    w1q_d = nc.dram_tensor("w1q", (P, DFF // P, D // 256, 2, P), F8,
                           kind="ExternalInput")
    # W2 packed per-output-chunk contiguous: [KO][P, DFF//256, 2, P]
    w2q_d = nc.dram_tensor("w2q", (KO, P, DFF // 256, 2, P), F8,
                           kind="ExternalInput")
    # packed per-partition biases: [bq | bk | b1 | b2] as [P, KO+KO+FO+KO]
    cpk_d = nc.dram_tensor("cpk", (P, 3 * KO + FO), F32, kind="ExternalInput")
    # rank-1 LN1-fold rows: [colsum(W1) ; b1] as [2, DFF] bf16
    c1r_d = nc.dram_tensor("c1r", (2, DFF), BF16, kind="ExternalInput")
    # output is written feature-major [D, T]; the host transposes
    out_d = nc.dram_tensor("out", (D, T), F32, kind="ExternalOutput")

    def wr(w):  # [K, M] weight dram -> [P, K//P, M] partition-chunked view
        return w[:, :].rearrange("(o p) m -> p o m", p=P)

    with tile.TileContext(nc) as tc:
        with (
            tc.tile_pool(name="sb", bufs=1) as sb,
            tc.tile_pool(name="ps", bufs=1, space="PSUM") as ps,
        ):
            # ---- phase A: projections; q first so the PE starts early ----
            KO2 = KO // 2
            xTq = sb.tile([P, KO2, 2, T], F8, tag="mid", bufs=2, name="xTq")
            qT = sb.tile([P, KO, T], BF16, tag="mid", bufs=2, name="qT")
            # PE warm-up: HAM releases the clock gate after ~3.4us of activity.
            # Junk matmuls on not-yet-written qT (never read back) span the
            # first input DMAs so the real chains start at full clock.
            wps = ps.tile([P, 2, T], F32, tag="mm", bufs=2, name="wps")
            for i in range(5):
                nc.tensor.matmul(wps[:, 0, :], lhsT=qT[:, 0, 0:P], rhs=qT[:, 0, :],
                                 start=(i == 0), stop=(i == 4))
            xT = sb.tile([P, KO2, 2, S], F8, tag="big", bufs=3, name="xT")
            kT = sb.tile([P, KO, S], BF16, tag="big", bufs=3, name="kT")
            vAug = sb.tile([P, TC, H, VW], BF16, tag="big", bufs=3, name="vAug")
            cpk = sb.tile([P, 3 * KO + FO], F32, name="cpk")
            bq_t, bk_t = cpk[:, 0:KO], cpk[:, KO:2 * KO]
            b2_t = cpk[:, 2 * KO + FO:]
            c1r_t = sb.tile([2, DFF], BF16, name="c1r_t")
            onesf_1p = sb.tile([1, P], F32, name="onesf_1p")
            onesf_bcol = sb.tile([P, 1], F32, name="onesf_bcol")
            ones32_1p = sb.tile([1, P], BF16, name="ones32_1p")
            ones_bcol = sb.tile([P, 1], BF16, name="ones_bcol")
            eps_t = sb.tile([1, 1], F32, name="eps_t")

            # startup DMAs: scalar (ACT) issues NO dmas — a dma_start occupies
            # its engine for the whole transfer and ACT is the exp bottleneck.
            # gpsimd (SWDGE) and sync (HWDGE) split the traffic, ordered by
            # first use.
            w4 = S // 4
            wq0 = sb.tile([P, KO2, 2, WS], F8, tag="wst", bufs=3, name="wq0")
            wq1 = sb.tile([P, KO2, 2, WS], F8, tag="wst", bufs=3, name="wq1")
            nc.gpsimd.dma_start(xTq, xTq_d[:, :, :, :])
            nc.sync.dma_start(wq0, Wq_d[:, :, :, 0:WS])
            nc.gpsimd.dma_start(cpk, cpk_d[:, :])
            nc.sync.dma_start(wq1, Wq_d[:, :, :, WS:2 * WS])
            nc.gpsimd.dma_start(xT[:, :, :, 0:w4], xT_d[:, :, :, 0:w4])
            nc.gpsimd.dma_start(xT[:, :, :, 2 * w4:3 * w4],
                                xT_d[:, :, :, 2 * w4:3 * w4])
            nc.vector.memset(onesf_1p, 1.0)
            nc.vector.memset(onesf_bcol, 1.0)
            nc.vector.memset(ones32_1p, 32.0)
            nc.vector.memset(ones_bcol, 1.0)
            nc.vector.memset(eps_t, EPS)
            # touch ACT immediately so its one-time table load (~2.7us with
            # drain) runs during the initial DMA wait instead of stalling the
            # first Q eviction
            nc.scalar.activation(eps_t, eps_t, AF.Copy, bias=0.0, scale=1.0)

            for mo2 in range(D // WS):
                wt = (wq0, wq1)[mo2]
                for mi in range(0, MI, 2):
                    pst = ps.tile([P, 2, T], F32, tag="mm", bufs=2, name=f"qp{mo2}_{mi}")
                    for half in range(2):
                        mo = mo2 * MI + mi + half
                        msl = slice((mi + half) * P, (mi + half + 1) * P)
                        for o2 in range(KO2):
                            nc.tensor.matmul(
                                pst[:, half, :],
                                lhsT=wt[:, o2, :, msl],
                                rhs=xTq[:, o2, :, :], perf_mode=DR,
                                start=(o2 == 0), stop=(o2 == KO2 - 1))
                        nc.scalar.activation(qT[:, mo, :], pst[:, half, :], AF.Identity,
                                             bias=bq_t[:, mo:mo + 1], scale=1.0 / 32)

            # k^T [D, S]: two token-chunk chains per psum tile, one batched evict
            for mo2 in range(D // WS):
                wt = sb.tile([P, KO2, 2, WS], F8, tag="wst", bufs=3, name=f"wk{mo2}")
                nc.sync.dma_start(wt, Wk_d[:, :, :, mo2 * WS:(mo2 + 1) * WS])
                if mo2 == 0:
                    nc.sync.dma_start(xT[:, :, :, w4:2 * w4],
                                      xT_d[:, :, :, w4:2 * w4])
                    nc.sync.dma_start(xT[:, :, :, 3 * w4:4 * w4],
                                      xT_d[:, :, :, 3 * w4:4 * w4])
                for mi in range(MI):
                    mo = mo2 * MI + mi
                    n_ch = S // SC
                    for nc2 in range((n_ch + 1) // 2):
                        w = min(2, n_ch - nc2 * 2)
                        pst = ps.tile([P, 2, SC], F32, tag="mm", bufs=2,
                                      name=f"kp{mo}_{nc2}")
                        for half in range(w):
                            ncc = nc2 * 2 + half
                            for o2 in range(KO2):
                                nc.tensor.matmul(pst[:, half, :],
                                                 lhsT=wt[:, o2, :, mi * P:(mi + 1) * P],
                                                 rhs=xT[:, o2, :, ncc * SC:(ncc + 1) * SC],
                                                 perf_mode=DR,
                                                 start=(o2 == 0), stop=(o2 == KO2 - 1))
                        nc.scalar.activation(
                            kT[:, mo, nc2 * 2 * SC:nc2 * 2 * SC + w * SC],
                            pst[:, 0:w, :], AF.Identity,
                            bias=bk_t[:, mo:mo + 1], scale=1.0 / 32)

            xres = sb.tile([P, KO, T], F32, tag="res", bufs=2, name="xres")

            # v token-major [S, D] with appended ones column per head:
            # vAug[p, tc, h, 0:64] = v[tc*128+p, h*64:(h+1)*64],  vAug[.., 64] = 1
            # (bv is folded into xres on the host: softmax weights sum to 1)
            # Evictions run on DVE so ACT stays exp-only during attention.
            nc.vector.memset(vAug[:, :, :, D_K:D_K + 1], 1.0)
            wv0 = sb.tile([P, KO2, 2, WS], F8, tag="wst", bufs=3, name="wv0")
            wv1 = sb.tile([P, KO2, 2, WS], F8, tag="wst", bufs=3, name="wv1")
            nc.sync.dma_start(wv0, Wv_d[:, :, :, 0:WS])
            nc.sync.dma_start(wv1, Wv_d[:, :, :, WS:2 * WS])
            # xres (2MB, first needed ~10us into attention) rides the SYNC
            # ring BEHIND all projection weights: engine DMA queues run ahead
            # of program order, so on gpsimd this transfer fired mid-K-phase
            # and starved the weight streams of aggregate SDMA bandwidth
            # (wk0 was observed arriving 17us after issue)
            nc.sync.dma_start(xres, xres_d[:, :].rearrange("(o p) t -> p o t", p=P))
            nh = WS // D_K

            def v_chunk(no2, tc_, wt):
                pfull = ps.tile([P, 2, SC], F32, tag="mm", bufs=2,
                                name=f"vp{no2}_{tc_}")
                pst = pfull[:, 0, :WS]
                for o2 in range(KO2):
                    nc.tensor.matmul(pst,
                                     lhsT=xT[:, o2, :, tc_ * P:(tc_ + 1) * P],
                                     rhs=wt[:, o2, :, :], perf_mode=DR,
                                     start=(o2 == 0), stop=(o2 == KO2 - 1))
                nc.scalar.activation(
                    vAug[:, tc_, no2 * nh:(no2 + 1) * nh, 0:D_K],
                    pst.rearrange("p (h d) -> p h d", d=D_K),
                    AF.Copy, bias=0.0, scale=1.0 / 32)

            for tc_ in range(TC):
                v_chunk(0, tc_, wv0)
            for tc_ in range(TC):
                v_chunk(1, tc_, wv1)

            # W1's 4MB preload is gated on xT's big-ring slot being released
            # by the last V chain, so it cannot contend with the startup
            nc.gpsimd.dma_start(c1r_t, c1r_d[:, :])
            w1q = sb.tile([P, DFF // P, D // 256, 2, P], F8, tag="big", bufs=3,
                          name="w1q")
            nc.gpsimd.dma_start(w1q, w1q_d[:, :, :, :])

            def bcast_prep(t):
                # stream_shuffle streams all 32 input lanes; zero the quadrant
                # BEFORE the row-0 write so nothing is read uninitialized.
                nc.vector.memset(t[0:32, :], 0.0)

            def bcast_from_row0(t, rows=128):
                """Replicate t[0:1, :] (SBUF) to partitions 0..rows, DVE-only."""
                nc.vector.stream_shuffle(t[32:64, :], t[0:32, :], mask=[0] * 32)
                nc.vector.tensor_copy(t[0:32, :], t[32:64, :])
                if rows > 64:
                    nc.vector.tensor_copy(t[64:96, :], t[32:64, :])
                    nc.vector.tensor_copy(t[96:128, :], t[32:64, :])

            # ---- phase B: attention with fused out-projection ----
            # Per kc chunk both heads of a pair land in ONE [P, 2, T] psum tile
            # so a single ACT exp covers 1024 elements. ctx row 64 = softmax
            # denominator (ones column of vAug). ctx accumulators live in a
            # 3-deep "cps" ring (pair p+1 never waits on pair p's normalize);
            # pair p's out-projection matmuls interleave into pair p+1's kc
            # loop against a dedicated 1-bank "op" ring, each eviction a DVE
            # add into xres.
            def norm_muls(hp, cpsA, cpsB):
                ctxp = sb.tile([P, T], BF16, tag="ctxp", bufs=2, name=f"cx{hp}")
                for h, cps in ((2 * hp, cpsA), (2 * hp + 1, cpsB)):
                    base = D_K * (h % 2)
                    bcs = sb.tile([P, T], F32, tag="scr", bufs=3, name=f"bc{h}")
                    bcast_prep(bcs)
                    # custom-DVE ops mis-read PSUM: stage the denominator row
                    # into SBUF first, then reciprocal in place.
                    nc.vector.tensor_copy(bcs[0:1, :], cps[D_K:D_K + 1, :])
                    nc.vector.reciprocal_approx_fast(bcs[0:1, :], bcs[0:1, :])
                    bcast_from_row0(bcs, rows=64)
                    nc.vector.tensor_mul(ctxp[base:base + D_K, :],
                                         cps[0:D_K, :], bcs[0:D_K, :])
                return ctxp

            def emit_op(hp, ctxp, wo_t, mo):
                # acc-ring op tile: during pair p's loop the ring holds the
                # two open ctx accumulators plus two free slots (pair p-1's,
                # released by its norm_muls at the top of pair p); the
                # allocator hands out free slots, so these never touch the
                # open accumulators or the score-tile (mm) ring.
                op = ps.tile([P, T], F32, tag="acc", bufs=4, name=f"o{hp}_{mo}")
                nc.tensor.matmul(op, lhsT=wo_t[:, mo * P:(mo + 1) * P],
                                 rhs=ctxp, start=True, stop=True)
                nc.vector.tensor_add(xres[:, mo, :], op, xres[:, mo, :])

            def op_pair(wo_t, ctxp, mg, name):
                """Two out-proj chunks into one mm-ring tile + one DVE add."""
                op2 = ps.tile([P, 2, T], F32, tag="mm", bufs=2, name=name)
                for half in range(2):
                    mo = 2 * mg + half
                    nc.tensor.matmul(op2[:, half, :],
                                     lhsT=wo_t[:, mo * P:(mo + 1) * P],
                                     rhs=ctxp, start=True, stop=True)
                xr2 = xres[:, 2 * mg:2 * mg + 2, :]
                nc.vector.tensor_add(xr2, op2, xr2)
                return xr2

            # Pair p-1's normalize (all-DVE) is emitted at pair p's top and
            # runs under the first ~6 exps; its 8 out-proj matmuls then
            # interleave one-per-2-kc from kc=6 (5 in-loop, 3 at the
            # boundary), filling PE slack in the ACT-bound steady state so
            # only pair 7's normalize+out-proj remains after the last exp.
            pend = None
            opnd = None
            for hp in range(HP):
                hA, hB = 2 * hp, 2 * hp + 1
                wo_t = sb.tile([P, D], BF16, tag="wo", bufs=2, name=f"wo{hp}")
                nc.sync.dma_start(wo_t, wr(Wo_d)[:, hp, :])
                cpsA = ps.tile([P, T], F32, tag="acc", bufs=4, name=f"cA{hp}")
                cpsB = ps.tile([P, T], F32, tag="acc", bufs=4, name=f"cB{hp}")
                if pend is not None:
                    opnd = (pend[0], norm_muls(*pend[:3]), pend[3])
                for kc in range(TC):
                    s2 = ps.tile([P, 2, T], F32, tag="mm", bufs=2, name=f"s{hp}_{kc}")
                    nc.tensor.matmul(s2[:, 0, :],
                                     lhsT=kT[0:D_K, hp, kc * P:(kc + 1) * P],
                                     rhs=qT[0:D_K, hp, :], start=True, stop=True)
                    nc.tensor.matmul(s2[:, 1, :],
                                     lhsT=kT[D_K:P, hp, kc * P:(kc + 1) * P],
                                     rhs=qT[D_K:P, hp, :], start=True, stop=True)
                    e2 = sb.tile([P, 2, T], BF16, tag="e", bufs=4,
                                 name=f"e{hp}_{kc}")
                    nc.scalar.activation(e2, s2, AF.Exp, scale=0.125)
                    nc.tensor.matmul(cpsA[0:D_K + 1, :],
                                     lhsT=vAug[:, kc, hA, 0:D_K + 1],
                                     rhs=e2[:, 0, :],
                                     start=(kc == 0), stop=(kc == TC - 1))
                    nc.tensor.matmul(cpsB[0:D_K + 1, :],
                                     lhsT=vAug[:, kc, hB, 0:D_K + 1],
                                     rhs=e2[:, 1, :],
                                     start=(kc == 0), stop=(kc == TC - 1))
                    if opnd is not None and ((kc >= 6 and kc % 2 == 0)
                                             or kc == 15):
                        j = 5 if kc == 15 else (kc - 6) // 2
                        emit_op(opnd[0], opnd[1], opnd[2], j)
                if opnd is not None:
                    for mo in (6, 7):
                        emit_op(opnd[0], opnd[1], opnd[2], mo)
                    opnd = None
                pend = (hp, cpsA, cpsB, wo_t)

            # ---- pair 7 out-projection with LN1 stats interleaved ----
            # No exp pressure remains: per 2 chunks, two op matmuls into one
            # mm-ring tile, ONE DVE add, ONE 1024-col fp8 quantize, then the
            # LN1 stat chains (LN1 itself is folded into FFN1).
            ctxp7, wo7 = norm_muls(7, cpsA, cpsB), wo_t
            xres_q = sb.tile([P, KO // 2, 2, T], F8, tag="mid", bufs=2, name="xres_q")
            sum1_ps = ps.tile([1, T], F32, tag="acc", bufs=4, name="sum1_ps")
            ssq1_ps = ps.tile([1, T], F32, tag="acc", bufs=4, name="ssq1_ps")
            for mg in range(KO // 2):
                xr2 = op_pair(wo7, ctxp7, mg, f"o7_{mg}")
                xq2 = xres_q[:, mg, :, :]
                nc.scalar.activation(xq2, xr2, AF.Copy, bias=0.0, scale=1.0)
                ysq = sb.tile([P, 2, T], BF16, tag="ysq", bufs=2, name=f"ys1_{mg}")
                nc.vector.tensor_mul(ysq, xq2, xq2)
                for half in range(2):
                    mo = 2 * mg + half
                    nc.tensor.matmul(sum1_ps, lhsT=ones_bcol, rhs=xq2[:, half, :],
                                     start=(mo == 0), stop=(mo == KO - 1))
                    nc.tensor.matmul(ssq1_ps, lhsT=ones_bcol, rhs=ysq[:, half, :],
                                     start=(mo == 0), stop=(mo == KO - 1))

            # ---- phase C: LN1 rows (stats only; LN1 is folded into FFN) ----
            # std via ACT Sqrt (one table switch, stays loaded through LN2),
            # rstd via the standard DVE reciprocal.
            mu1f = sb.tile([1, T], F32, tag="lns", bufs=4, name="mu1f")
            nc.scalar.activation(mu1f, sum1_ps, AF.Copy, bias=0.0, scale=1.0 / D)
            var1 = sb.tile([1, T], F32, tag="lns", bufs=4, name="var1")
            nc.vector.tensor_mul(var1, mu1f, mu1f)
            nc.vector.scalar_tensor_tensor(out=var1, in0=ssq1_ps, scalar=1.0 / D,
                                           in1=var1, op0=OP.mult, op1=OP.subtract)
            std1 = sb.tile([1, T], F32, tag="lns", bufs=4, name="std1")
            nc.scalar.activation(std1, var1, AF.Sqrt, bias=eps_t[0:1, 0:1],
                                 scale=1.0)
            rstd1_sb = sb.tile([P, T], F32, tag="scr", bufs=3, name="rstd1_sb")
            bcast_prep(rstd1_sb)
            nc.vector.reciprocal(rstd1_sb[0:1, :], std1)
            bcast_from_row0(rstd1_sb)
            # pack [-mu ; std] contiguously in one partition-0 stage tile and
            # ship both rows with a single partition-crossing DMA (engine APs
            # can't write partition 1, but DMA descriptors can)
            rstage = sb.tile([1, 2, T], BF16, tag="lns", bufs=4, name="rstage")
            nc.scalar.activation(rstage[0:1, 0, :], sum1_ps, AF.Copy, bias=0.0,
                                 scale=-1.0 / D)
            nc.scalar.activation(rstage[0:1, 1, :], std1, AF.Copy, bias=0.0,
                                 scale=1.0)
            negmu_row = rstage[0:1, 0, :]
            rmix = sb.tile([2, T], BF16, tag="lns", bufs=4, name="rmix")
            # two plain partition-targeted DMAs: the single partition-
            # expanding transfer was observed to race its completion
            # semaphore (stride-4 column groups landing late)
            nc.sync.dma_start(rmix[0:1, :], rstage[0:1, 0, :])
            nc.sync.dma_start(rmix[1:2, :], rstage[0:1, 1, :])

            # ---- phase D: FFN1 + relu (std-scaled domain, no bias on ACT) ----
            # fp8 DoubleRow: 4 K=256 matmuls per chain; psum carries 32x the
            # true value (host-scaled weights), rescaled at the relu eviction.
            rT = sb.tile([P, FO // 2, 2, T], F8, tag="big", bufs=3, name="rT")
            for fo2 in range(DFF // WS):
                for fi in range(0, MI, 2):
                    pst = ps.tile([P, 2, T], F32, tag="mm", bufs=2, name=f"zp{fo2}_{fi}")
                    # run both halves' DR passes as ONE 8-pass fp8 streak and
                    # append the two bf16 rank-1 folds after: each DR<->normal
                    # perf-mode switch costs LDW-pipeline refill, so two
                    # switches per tile instead of four
                    for half in range(2):
                        fo = fo2 * MI + fi + half
                        for o2 in range(KO // 2):
                            nc.tensor.matmul(pst[:, half, :],
                                             lhsT=w1q[:, fo, o2, :, :],
                                             rhs=xres_q[:, o2, :, :],
                                             perf_mode=DR,
                                             start=(o2 == 0), stop=False)
                    for half in range(2):
                        fo = fo2 * MI + fi + half
                        # K=2 rank-1 fold: (-mu)(x)colsum(W1) + std(x)b1
                        nc.tensor.matmul(pst[:, half, :],
                                         lhsT=c1r_t[0:2, fo * P:(fo + 1) * P],
                                         rhs=rmix[0:2, :], start=False, stop=True)
                    # both halves land in the same rT [., fo//2, 0:2, .]
                    # slice: ONE batched 1024-col relu eviction per psum tile
                    fo = fo2 * MI + fi
                    nc.scalar.activation(rT[:, fo // 2, :, :], pst,
                                         AF.Relu, bias=0.0, scale=1.0 / 32)

            # ---- phase E: FFN2 + residual + LN2 stats (interleaved) ----
            y2 = sb.tile([P, KO, T], F32, tag="res", bufs=2, name="y2")
            sum2_ps = ps.tile([1, T], F32, tag="acc", bufs=4, name="sum2_ps")
            ssq2_ps = ps.tile([1, T], F32, tag="acc", bufs=4, name="ssq2_ps")
            for mo in range(KO):
                pfull = ps.tile([P, 2, T], F32, tag="mm", bufs=2, name=f"fp{mo}")
                pst = pfull[:, 0, :]
                w2t = sb.tile([P, DFF // 256, 2, P], F8, tag="w2", bufs=2,
                              name=f"w2_{mo}")
                nc.sync.dma_start(w2t, w2q_d[mo])
                for ki in range(DFF // 256):
                    nc.tensor.matmul(pst, lhsT=w2t[:, ki, :, :],
                                     rhs=rT[:, ki, :, :], perf_mode=DR,
                                     start=(ki == 0), stop=False)
                # rank-1: subtract 32*mu1 (broadcast over features) in-psum
                nc.tensor.matmul(pst, lhsT=ones32_1p[0:1, :],
                                 rhs=negmu_row, start=False, stop=True)
                # y2 = rstd1*(ffpsum/32 + xres - mu1) + b2
                nc.vector.scalar_tensor_tensor(out=y2[:, mo, :], in0=pst,
                                               scalar=1.0 / 32, in1=xres[:, mo, :],
                                               op0=OP.mult, op1=OP.add)
                nc.vector.tensor_mul(y2[:, mo, :], y2[:, mo, :], rstd1_sb)
                nc.vector.tensor_scalar_add(y2[:, mo, :], y2[:, mo, :],
                                            b2_t[:, mo:mo + 1])
                # LN2 stats accumulate as chunks complete; the sum chain
                # consumes y2 (f32) directly — no bf16 staging copy
                ysq2 = sb.tile([P, T], BF16, tag="ysq", bufs=2, name=f"ys2_{mo}")
                nc.scalar.activation(ysq2, y2[:, mo, :], AF.Square, bias=0.0,
                                     scale=1.0)
                nc.tensor.matmul(sum2_ps, lhsT=onesf_bcol, rhs=y2[:, mo, :],
                                 start=(mo == 0), stop=(mo == KO - 1))
                nc.tensor.matmul(ssq2_ps, lhsT=ones_bcol, rhs=ysq2,
                                 start=(mo == 0), stop=(mo == KO - 1))

            # ---- phase F: LN2 normalize feature-major + store ----
            # -mu2 and rstd2 rows are PE-broadcast to all 128 partitions via
            # rank-1 matmuls into PSUM; the normalize is then two DVE
            # tensor_tensor ops per chunk and the output DMAs feature-major
            # (the host transposes).
            negmu2 = sb.tile([1, T], F32, tag="lns", bufs=4, name="negmu2")
            nc.scalar.activation(negmu2, sum2_ps, AF.Copy, bias=0.0,
                                 scale=-1.0 / D)
            t2m = sb.tile([1, T], F32, tag="lns", bufs=4, name="t2m")
            nc.vector.tensor_mul(t2m, negmu2, negmu2)
            var2 = sb.tile([1, T], F32, tag="lns", bufs=4, name="var2")
            nc.vector.scalar_tensor_tensor(out=var2, in0=ssq2_ps, scalar=1.0 / D,
                                           in1=t2m, op0=OP.mult, op1=OP.subtract)
            std2 = sb.tile([1, T], F32, tag="lns", bufs=4, name="std2")
            nc.scalar.activation(std2, var2, AF.Sqrt, bias=eps_t[0:1, 0:1],
                                 scale=1.0)
            rstd2 = sb.tile([1, T], F32, tag="lns", bufs=4, name="rstd2")
            nc.vector.reciprocal(rstd2, std2)
            nm2_ps = ps.tile([P, T], F32, tag="acc", bufs=4, name="nm2_ps")
            rs2_ps = ps.tile([P, T], F32, tag="acc", bufs=4, name="rs2_ps")
            nc.tensor.matmul(nm2_ps, lhsT=onesf_1p, rhs=negmu2,
                             start=True, stop=True)
            nc.tensor.matmul(rs2_ps, lhsT=onesf_1p, rhs=rstd2,
                             start=True, stop=True)
            out_r = out_d[:, :].rearrange("(o p) t -> p o t", p=P)
            for mo in range(KO):
                y2n = sb.tile([P, T], F32, tag="scr", bufs=3, name=f"y2n_{mo}")
                nc.vector.tensor_add(y2n, y2[:, mo, :], nm2_ps)
                nc.vector.tensor_mul(y2n, y2n, rs2_ps)
                # scalar (ACT) is idle at the tail; both HWDGE queues share
                # the output so the last chunk lands as early as possible
                (nc.sync, nc.scalar)[mo % 2].dma_start(out_r[:, mo, :], y2n)

    nc.finalize()
    return nc


def _maybe_enable_ldw_opt():
    if os.environ.get("BASS_LDW_OPT") != "1":
        return
    import concourse.bass_utils as _bu
    if getattr(_bu, "_ldw_opt_patched", False):
        return
    _orig = _bu.run_command

    def _patched(argv, **kw):
        argv = ["--enable-ldw-opt=true" if a == "--enable-ldw-opt=false" else a
                for a in argv]
        return _orig(argv, **kw)

    _bu.run_command = _patched
    _bu._ldw_opt_patched = True


_maybe_enable_ldw_opt()

_PROG = None
_last_results = None


def _get_prog():
    global _PROG
    if _PROG is None:
        _PROG = build_program()
    return _PROG


def pack_consts(bq, bk, b1, b2, KO=D_MODEL // P, FO=D_FF // P):
    cols = []
    for vec, n in ((bq, KO), (bk, KO), (b1, FO), (b2, KO)):
        cols.append(np.asarray(vec, np.float32).reshape(n, P).T)  # [P, n]
    return np.ascontiguousarray(np.concatenate(cols, axis=1))


def make_in_maps(x, Wq, bq, Wk, bk, Wv, bv, Wo, bo, W1, b1, W2, b2,
                 ln1_g, ln1_b, ln2_g, ln2_b):
    bf = ml_dtypes.bfloat16
    f32 = np.float32
    x = np.asarray(x, f32)
    f8 = ml_dtypes.float8_e4m3
    W1f = np.asarray(W1, f32)
    c1r = (32.0 * np.stack([W1f.sum(axis=0), np.asarray(b1, f32)])).astype(bf)

    def pack_dr(w):  # [K, M] -> [P, K//256, 2, M] fp8, pre-scaled by 32
        K, M = w.shape
        wi = (np.asarray(w, f32) * 32.0).reshape(K // 256, 2, P, M)
        return np.ascontiguousarray(wi.transpose(2, 0, 1, 3).astype(f8))

    w2q = pack_dr(np.asarray(W2, f32))          # [P, DFF//256, 2, D]
    # repack per-output-chunk contiguous: [KO, P, DFF//256, 2, P]
    w2q = np.ascontiguousarray(
        w2q.reshape(P, D_FF // 256, 2, D_MODEL // P, P).transpose(3, 0, 1, 2, 4))

    shared = {
        "Wq": pack_dr(np.asarray(Wq, f32)),
        "Wk": pack_dr(np.asarray(Wk, f32)),
        "Wv": pack_dr(np.asarray(Wv, f32)),
        "Wo": np.ascontiguousarray(np.asarray(Wo, f32).astype(bf)),
        "w1q": np.ascontiguousarray(
            pack_dr(W1f).reshape(P, D_MODEL // 256, 2, D_FF // P, P)
            .transpose(0, 3, 1, 2, 4)),
        "w2q": w2q,
        "cpk": pack_consts(bq, bk, b1, b2),
        "c1r": np.ascontiguousarray(c1r),
    }
    # bv is invariant under softmax averaging: attn(v + bv) = attn(v) + bv,
    # so fold bv@Wo + bo into the residual once on the host (exact, f32).
    res_bias = (np.asarray(bv, f32) @ np.asarray(Wo, f32)
                + np.asarray(bo, f32))

    def pack_act(a):  # [D, Ntok] -> [P, D//256, 2, Ntok] fp8 interleaved
        Dd, Nt = a.shape
        return np.ascontiguousarray(
            a.reshape(Dd // 256, 2, P, Nt).transpose(2, 0, 1, 3).astype(f8))

    in_maps = []
    xT_by_batch = [np.ascontiguousarray(x[b].T) for b in range(x.shape[0])]
    xTq_by_batch = [pack_act(t) for t in xT_by_batch]
    for c in range(N_CORES):
        b, q0 = c // 4, (c % 4) * TQ
        xslice = xT_by_batch[b][:, q0:q0 + TQ]
        m = dict(shared)
        m["xT"] = xTq_by_batch[b]
        m["xTq"] = np.ascontiguousarray(xTq_by_batch[b][:, :, :, q0:q0 + TQ])
        m["xres"] = np.ascontiguousarray(xslice + res_bias[:, None])
        in_maps.append(m)
    return in_maps


def kernel(**inputs):
    global _last_results
    nc = _get_prog()
    in_maps = make_in_maps(**inputs)
    res = run_bass_kernel_spmd(nc, in_maps, core_ids=list(range(N_CORES)),
                               tmpdir=os.environ.get("BASS_KERNEL_TMPDIR"))
    _last_results = res
    x = np.asarray(inputs["x"])
    B, S, D = x.shape
    out = np.empty((B, S, D), np.float32)
    for c in range(N_CORES):
        b, q0 = c // 4, (c % 4) * TQ
        out[b, q0:q0 + TQ, :] = res.results[c]["out"].T
    return out
